# revision 1
# baseline (speedup 1.0000x reference)
"""CGCNN (gnn_message_passing) Trainium2 kernel — 8-core SPMD.

Strategy:
  - Nodes partitioned contiguously across 8 cores (6250/core, padded to 6272);
    edges assigned to the core owning their dst node, sorted by dst, grouped
    into 128-edge chunks that never cross a 128-node dst block (host padding,
    pad edges read an all-zero table row so they contribute exactly 0).
  - Per conv layer each core computes projection tables
      A_src = v @ [Wm_src|Ws_src]  (AllGathered; gathered per edge by src via
                                    dma_gather over 4 SWDGE queues)
      A_dst = v @ [Wm_dst|Ws_dst]  (local DRAM; gathered per edge by dst)
    z[e] = A_src[src] + A_dst[dst] + ef[e] @ Wef  (ef-projection via matmul of
    pre-transposed edge features; adds are group-wide vector ops).
  - BatchNorm is exact: pass 1 spills z to DRAM and accumulates sum/sumsq in
    on-chip accumulators (pads are exact zeros), tiny AllReduce; pass 2
    reloads z, applies folded BN affine + sigmoid/softplus (built from
    Exp/Ln/Abs/Relu so one activation table serves the whole kernel) and
    scatter-sums h into the local agg block via per-chunk indicator matmuls
    (dst-block index read into a register for the dynamic accumulate).
  - Node BN: local sums + tiny AllReduce. Readout (per-graph mean + 2 MLPs +
    head) computed redundantly per core via graph-indicator matmuls + one
    small AllReduce.  Linear biases feeding BN cancel and are ignored.
"""

import sys
import os
from contextlib import ExitStack

sys.path.insert(0, "/opt/trn_rl_repo")

import numpy as np

import concourse.bass as bass
import concourse.bacc as bacc
import concourse.tile as tile
from concourse import mybir, bass_utils
import concourse.hw_specs as hw_specs

FP = mybir.dt.float32

# Restrict activation-table selection to one set so the scalar engine never
# reloads tables (everything is built from Exp/Ln/Abs/Relu/Identity/Copy).
_KEEP_TABLES = {"natural_log_exp_and_others"}


def _patched_tables(arch):
    t = hw_specs.get_activation_tables(arch)
    return {k: (v if k in _KEEP_TABLES else set()) for k, v in t.items()}


bacc.get_activation_tables = _patched_tables


# ---------------------------------------------------------------- config
class Cfg:
    def __init__(self, N, M, NG):
        self.NC = 8
        self.N, self.M, self.NG = N, M, NG
        self.FV, self.FE, self.E, self.L = 92, 41, 64, 3
        self.FC0, self.FC1 = 128, 64
        self.ZF = 128                       # z width = 2*E
        self.NB = N // self.NC              # real nodes per core
        self.NBP = -(-(self.NB + 1) // 128) * 128  # padded (>= NB+1: zero row)
        self.NBLK = self.NBP // 128
        self.NT = self.NBP * self.NC
        self.HALF = self.NT // 2
        assert self.HALF - 1 < 32768
        assert self.NBP > self.NB
        self.GS = 16                        # chunks per group (2048 edges)
        self.EPS = 1e-5


# ---------------------------------------------------------- preprocessing
def _wrap_idx16(idx):
    a = idx.reshape(-1, 16).T.astype(np.int16)
    return np.tile(a, (8, 1))


def preprocess(inputs, cfg):
    c = cfg
    src = np.asarray(inputs["src"]).astype(np.int64)
    dst = np.asarray(inputs["dst"]).astype(np.int64)
    ef = np.asarray(inputs["edge_feats"], np.float32)
    nf = np.asarray(inputs["node_feats"], np.float32)
    gid = np.asarray(inputs["graph_ids"]).astype(np.int64)

    pad_row = (src // c.NB) * c.NBP + (src % c.NB)
    owner = dst // c.NB
    dst_loc = dst - owner * c.NB

    cores = []
    for core in range(c.NC):
        em = np.nonzero(owner == core)[0]
        bucket = (pad_row[em] >= c.HALF).astype(np.int64)
        per_bucket = []
        for b in (0, 1):
            eb = em[bucket == b]
            eb = eb[np.argsort(dst_loc[eb], kind="stable")]
            blk = dst_loc[eb] // 128
            segs = []
            for bk in range(c.NBLK):
                run = eb[blk == bk]
                segs.append((run, bk, (-len(run)) % 128))
            per_bucket.append(segs)
        cores.append(per_bucket)

    EP = [0, 0]
    for b in (0, 1):
        for core in range(c.NC):
            tot = sum(len(r) + p for r, _, p in cores[core][b])
            EP[b] = max(EP[b], tot)
        EP[b] = max(-(-EP[b] // 128) * 128, 128)
    EPT = EP[0] + EP[1]
    ZROW = c.NB  # all-zero table row (first pad node), same rel id both halves

    in_maps = []
    for core in range(c.NC):
        srcrel = np.full(EPT, ZROW, np.int64)
        dstrel = np.full(EPT, ZROW, np.int64)
        dstblk = np.full(EPT, -1.0, np.float32)
        blkid = np.zeros(EPT // 128, np.int32)
        eperm = np.full(EPT, -1, np.int64)
        for b in (0, 1):
            boff = b * EP[0]
            pos = 0
            for run, bk, npad in cores[core][b]:
                n = len(run)
                if n:
                    sl = slice(boff + pos, boff + pos + n)
                    srcrel[sl] = pad_row[run] - b * c.HALF
                    dstrel[sl] = dst_loc[run]
                    dstblk[sl] = (dst_loc[run] - bk * 128).astype(np.float32)
                    eperm[sl] = run
                blkid[(boff + pos) // 128: (boff + pos + n + npad) // 128] = bk
                pos += n + npad

        eft = np.zeros((c.FE, EPT), np.float32)
        real = eperm >= 0
        eft[:, real] = ef[eperm[real]].T

        nfT = np.zeros((c.FV, c.NBP), np.float32)
        nfT[:, : c.NB] = nf[core * c.NB: (core + 1) * c.NB].T
        gidc = np.full(c.NBP, -1.0, np.float32)
        gidc[: c.NB] = gid[core * c.NB: (core + 1) * c.NB].astype(np.float32)

        eye = np.eye(129, 128, dtype=np.float32)
        bidx = np.where(dstblk < 0, 128, dstblk.astype(np.int64))
        indt = eye[bidx].reshape(-1, 128, 128)          # [NCH, 128e, 128d]
        m = {
            "srcrel": _wrap_idx16(srcrel.astype(np.int16)),
            "dstrel": _wrap_idx16(dstrel.astype(np.int16)),
            "indt": indt,
            "blkid": blkid.reshape(1, -1),
            "eft": eft,
            "nfT": nfT,
            "gidc": gidc.reshape(-1, 128).T.copy(),
        }
        in_maps.append(m)

    Wm = np.asarray(inputs["Wm"], np.float32)
    Ws = np.asarray(inputs["Ws"], np.float32)
    E = c.E
    shared = {
        "W_emb": np.asarray(inputs["W_emb"], np.float32),
        "g_emb": np.asarray(inputs["g_emb"], np.float32).reshape(1, E),
        "be_emb": np.asarray(inputs["be_emb"], np.float32).reshape(1, E),
        "Wsrc2": np.concatenate([Wm[:, :E, :], Ws[:, :E, :]], axis=2),
        "Wdst2": np.concatenate([Wm[:, E:2 * E, :], Ws[:, E:2 * E, :]], axis=2),
        "Wef2": np.concatenate([Wm[:, 2 * E:, :], Ws[:, 2 * E:, :]], axis=2),
        "gm": np.asarray(inputs["gm"], np.float32),
        "bem": np.asarray(inputs["bem"], np.float32),
        "gs": np.asarray(inputs["gs"], np.float32),
        "bes": np.asarray(inputs["bes"], np.float32),
        "gn": np.asarray(inputs["gn"], np.float32),
        "ben": np.asarray(inputs["ben"], np.float32),
        "Wf0": np.asarray(inputs["Wf0"], np.float32),
        "gf0": np.asarray(inputs["gf0"], np.float32).reshape(-1, 1),
        "bef0": np.asarray(inputs["bef0"], np.float32).reshape(-1, 1),
        "Wf1": np.asarray(inputs["Wf1"], np.float32),
        "gf1": np.asarray(inputs["gf1"], np.float32).reshape(-1, 1),
        "bef1": np.asarray(inputs["bef1"], np.float32).reshape(-1, 1),
        "Wt": np.asarray(inputs["Wt"], np.float32),
        "bt": np.asarray(inputs["bt"], np.float32).reshape(1, 1),
    }
    for m in in_maps:
        m.update(shared)
    return in_maps, EP


# ------------------------------------------------------------- kernel build
def build(cfg, EP):
    c = cfg
    EPT = EP[0] + EP[1]
    NCH = EPT // 128
    DVE = mybir.EngineType.DVE
    AF = mybir.ActivationFunctionType
    OP = mybir.AluOpType

    nc = bacc.Bacc("TRN2", target_bir_lowering=False, debug=False,
                   enable_asserts=False, num_devices=c.NC, num_swdge_queues=4)

    def din(name, shape, dt=FP):
        return nc.dram_tensor(name, shape, dt, kind="ExternalInput")

    t_srcrel = din("srcrel", [128, EPT // 16], mybir.dt.int16)
    t_dstrel = din("dstrel", [128, EPT // 16], mybir.dt.int16)
    t_indt = din("indt", [NCH, 128, 128])
    t_blkid = din("blkid", [1, NCH], mybir.dt.int32)
    t_eft = din("eft", [c.FE, EPT])
    t_nfT = din("nfT", [c.FV, c.NBP])
    t_gidc = din("gidc", [128, c.NBLK])
    t_Wemb = din("W_emb", [c.FV, c.E])
    t_gemb = din("g_emb", [1, c.E])
    t_beemb = din("be_emb", [1, c.E])
    t_Wsrc2 = din("Wsrc2", [c.L, c.E, c.ZF])
    t_Wdst2 = din("Wdst2", [c.L, c.E, c.ZF])
    t_Wef2 = din("Wef2", [c.L, c.FE, c.ZF])
    t_gm = din("gm", [c.L, c.E])
    t_bem = din("bem", [c.L, c.E])
    t_gs = din("gs", [c.L, c.E])
    t_bes = din("bes", [c.L, c.E])
    t_gn = din("gn", [c.L, c.E])
    t_ben = din("ben", [c.L, c.E])
    t_Wf0 = din("Wf0", [c.E, c.FC0])
    t_gf0 = din("gf0", [c.FC0, 1])
    t_bef0 = din("bef0", [c.FC0, 1])
    t_Wf1 = din("Wf1", [c.FC0, c.FC1])
    t_gf1 = din("gf1", [c.FC1, 1])
    t_bef1 = din("bef1", [c.FC1, 1])
    t_Wt = din("Wt", [c.E, 1])
    t_bt = din("bt", [1, 1])
    t_out = nc.dram_tensor("out", [1, c.NG], FP, kind="ExternalOutput")

    RG = [list(range(c.NC))]

    with tile.TileContext(nc) as tc, ExitStack() as es:
        dram = es.enter_context(tc.tile_pool(name="dram", bufs=1, space="DRAM"))
        zbuf = dram.tile([128, NCH, c.ZF], FP)
        adst_dram = dram.tile([c.NBP, c.ZF], FP)
        est_in = [dram.tile([1, 2 * c.ZF], FP, name=f"est_in{i}") for i in range(c.L)]
        est_out = [dram.tile([1, 2 * c.ZF], FP, addr_space="Shared", name=f"est_out{i}")
                   for i in range(c.L)]
        nst_in = [dram.tile([1, 2 * c.E], FP, name=f"nst_in{i}") for i in range(c.L + 1)]
        nst_out = [dram.tile([1, 2 * c.E], FP, addr_space="Shared", name=f"nst_out{i}")
                   for i in range(c.L + 1)]
        agin_l = [dram.tile([c.NBP, c.ZF], FP, name=f"agin{i}") for i in range(c.L)]
        agout_l = [dram.tile([c.NT, c.ZF], FP, addr_space="Shared", name=f"agout{i}")
                   for i in range(c.L)]
        ro_in = dram.tile([c.E + 1, c.NG], FP)
        ro_out = dram.tile([c.E + 1, c.NG], FP, addr_space="Shared")

        konst = es.enter_context(tc.tile_pool(name="konst", bufs=1))
        iotaF = konst.tile([128, 256], FP)
        identF = konst.tile([128, 128], FP)
        ones_row = konst.tile([1, 128], FP)
        ones_col = konst.tile([128, 1], FP)
        epsT = konst.tile([1, 1], FP)
        epsC = konst.tile([128, 1], FP)
        padmask = konst.tile([128, 1], FP)
        with tc.tile_pool(name="ksetup", bufs=1) as ks:
            ii = ks.tile([128, 256], mybir.dt.int32)
            nc.gpsimd.iota(ii[:], pattern=[[1, 256]], base=0, channel_multiplier=0)
            nc.vector.tensor_copy(iotaF[:], ii[:])
            ip = ks.tile([128, 1], mybir.dt.int32)
            nc.gpsimd.iota(ip[:], pattern=[[1, 1]], base=0, channel_multiplier=1)
            ipf = ks.tile([128, 1], FP)
            nc.vector.tensor_copy(ipf[:], ip[:])
            nc.vector.tensor_scalar(identF[:], iotaF[:, :128], ipf[:], None, OP.is_equal)
            nc.vector.tensor_scalar(padmask[:], ipf[:], float(c.NB % 128), None, OP.is_lt)
        nc.vector.memset(ones_row[:], 1.0)
        nc.vector.memset(ones_col[:], 1.0)
        nc.vector.memset(epsT[:], c.EPS)
        nc.vector.memset(epsC[:], c.EPS)

        state = es.enter_context(tc.tile_pool(name="state", bufs=1))
        v_sb = state.tile([128, c.NBLK, c.E], FP)
        agg_sb = state.tile([128, c.NBLK, c.E], FP)
        blkid_sb = state.tile([1, NCH], mybir.dt.int32)
        gid_sb = state.tile([128, c.NBLK], FP)
        nc.sync.dma_start(blkid_sb[:], t_blkid[:])
        nc.sync.dma_start(gid_sb[:], t_gidc[:])

        wts = es.enter_context(tc.tile_pool(name="wts", bufs=1))
        Wsrc2_sb = wts.tile([c.E, c.L * c.ZF], FP)
        Wdst2_sb = wts.tile([c.E, c.L * c.ZF], FP)
        Wef2_sb = wts.tile([c.FE, c.L * c.ZF], FP)
        for l in range(c.L):
            nc.sync.dma_start(Wsrc2_sb[:, l * c.ZF:(l + 1) * c.ZF], t_Wsrc2[l])
            nc.sync.dma_start(Wdst2_sb[:, l * c.ZF:(l + 1) * c.ZF], t_Wdst2[l])
            nc.sync.dma_start(Wef2_sb[:, l * c.ZF:(l + 1) * c.ZF], t_Wef2[l])

        # sigmoid(x) -> out, via one act table: sig = exp(-softplus(-x))
        def sigmoid_ops(pool, out, x, shape, nm):
            t1 = pool.tile(shape, FP, name=f"sgA{nm}", tag=f"sgA{nm}")
            nc.scalar.activation(t1[:], x, AF.Abs)
            nc.scalar.activation(t1[:], t1[:], AF.Exp, scale=-1.0)
            nc.any.tensor_scalar_add(t1[:], t1[:], 1.0)
            nc.scalar.activation(t1[:], t1[:], AF.Ln)
            t2 = pool.tile(shape, FP, name=f"sgB{nm}", tag=f"sgB{nm}")
            nc.vector.tensor_scalar(t2[:], x, 0.0, -1.0, OP.min, OP.mult)
            nc.any.tensor_add(t1[:], t1[:], t2[:])
            nc.scalar.activation(out, t1[:], AF.Exp, scale=-1.0)

        # softplus(x) -> out = ln(1+exp(-|x|)) + relu(x)
        def softplus_ops(pool, out, x, shape, nm):
            t1 = pool.tile(shape, FP, name=f"spA{nm}", tag=f"spA{nm}")
            nc.scalar.activation(t1[:], x, AF.Abs)
            nc.scalar.activation(t1[:], t1[:], AF.Exp, scale=-1.0)
            nc.any.tensor_scalar_add(t1[:], t1[:], 1.0)
            nc.scalar.activation(t1[:], t1[:], AF.Ln)
            t2 = pool.tile(shape, FP, name=f"spB{nm}", tag=f"spB{nm}")
            nc.scalar.activation(t2[:], x, AF.Relu)
            nc.any.tensor_add(out, t1[:], t2[:])

        def bn_fold(pool, sums, F, count, g_ap, be_ap):
            st = pool.tile([1, 2 * F], FP, name=f"bnf{nc.next_id()}")
            mean = pool.tile([1, F], FP, name=f"bnm{nc.next_id()}")
            var = pool.tile([1, F], FP, name=f"bnv{nc.next_id()}")
            nc.scalar.mul(mean[:], sums[:, 0:F], 1.0 / count)
            nc.scalar.mul(var[:], sums[:, F:2 * F], 1.0 / count)
            m2 = pool.tile([1, F], FP, name=f"bn2{nc.next_id()}")
            nc.vector.tensor_mul(m2[:], mean[:], mean[:])
            nc.vector.tensor_sub(var[:], var[:], m2[:])
            nc.scalar.activation(var[:], var[:], AF.Ln, bias=epsT[0:1, 0:1])
            nc.scalar.activation(var[:], var[:], AF.Exp, scale=-0.5)
            nc.vector.tensor_mul(st[:, 0:F], g_ap, var[:])
            nc.vector.tensor_mul(mean[:], mean[:], st[:, 0:F])
            nc.vector.tensor_sub(st[:, F:2 * F], be_ap, mean[:])
            return st

        def bcast_row(pool, psum_pool, row_ap, W, name):
            ps = psum_pool.tile([128, W], FP, name=f"ps{name}")
            nc.tensor.matmul(ps[:], ones_row[:, :], row_ap, start=True, stop=True)
            sb = pool.tile([128, W], FP, name=name)
            nc.scalar.copy(sb[:], ps[:])
            return sb

        def zero_vpad():
            # zero pad-node rows of the last block (per-partition mask multiply)
            cb = c.NB // 128
            nc.vector.tensor_scalar(v_sb[:, cb, :], v_sb[:, cb, :],
                                    padmask[:], None, OP.mult)

        # ---------------------------------------------------- embedding
        with tc.tile_pool(name="emb", bufs=1) as emb, \
             tc.tile_pool(name="embw", bufs=2) as embw, \
             tc.tile_pool(name="embp", bufs=2, space="PSUM") as embp, \
             tc.tile_pool(name="embs", bufs=1, space="PSUM") as embs:
            nfT_sb = emb.tile([c.FV, c.NBP], FP)
            nc.sync.dma_start(nfT_sb[:], t_nfT[:])
            Wemb_sb = emb.tile([c.FV, c.E], FP)
            nc.sync.dma_start(Wemb_sb[:], t_Wemb[:])
            z0 = emb.tile([128, c.NBLK, c.E], FP)
            ssum = embs.tile([1, c.E], FP)
            ssq = embs.tile([1, c.E], FP)
            for ch in range(c.NBLK):
                ps = embp.tile([128, c.E], FP, name="embz")
                nc.tensor.matmul(ps[:], nfT_sb[:, ch * 128:(ch + 1) * 128],
                                 Wemb_sb[:], start=True, stop=True)
                nc.scalar.copy(z0[:, ch, :], ps[:])
                sq = embw.tile([128, c.E], FP, name="embsq")
                nc.vector.tensor_mul(sq[:], z0[:, ch, :], z0[:, ch, :])
                nc.tensor.matmul(ssum[:], ones_col[:, :], z0[:, ch, :],
                                 start=(ch == 0), stop=(ch == c.NBLK - 1))
                nc.tensor.matmul(ssq[:], ones_col[:, :], sq[:],
                                 start=(ch == 0), stop=(ch == c.NBLK - 1))
            stat = emb.tile([1, 2 * c.E], FP)
            nc.vector.tensor_copy(stat[:, 0:c.E], ssum[:])
            nc.vector.tensor_copy(stat[:, c.E:], ssq[:])
            nc.sync.dma_start(nst_in[c.L][:], stat[:])
            nc.gpsimd.collective_compute(
                "AllReduce", OP.add, replica_groups=RG,
                ins=[nst_in[c.L].opt()], outs=[nst_out[c.L].opt()])
            rstat = emb.tile([1, 2 * c.E], FP)
            nc.sync.dma_start(rstat[:], nst_out[c.L][:])
            gemb_sb = emb.tile([1, c.E], FP)
            beemb_sb = emb.tile([1, c.E], FP)
            nc.sync.dma_start(gemb_sb[:], t_gemb[:])
            nc.sync.dma_start(beemb_sb[:], t_beemb[:])
            st = bn_fold(emb, rstat, c.E, c.N, gemb_sb[:], beemb_sb[:])
            stb = bcast_row(emb, embp, st[:], 2 * c.E, "embst")
            for ch in range(c.NBLK):
                u = embw.tile([128, c.E], FP, name="embu")
                nc.vector.tensor_mul(u[:], z0[:, ch, :], stb[:, 0:c.E])
                nc.vector.tensor_add(u[:], u[:], stb[:, c.E:])
                sg = embw.tile([128, c.E], FP, name="embsg")
                sigmoid_ops(embw, sg[:], u[:], [128, c.E], "emb")
                nc.vector.tensor_mul(v_sb[:, ch, :], u[:], sg[:])
            zero_vpad()

        # ---------------------------------------------------- conv layers
        gq = 0
        for l in range(c.L):
            # ---- phase A: projection tables
            with tc.tile_pool(name="phA", bufs=2) as pa, \
                 tc.tile_pool(name="phAp", bufs=2, space="PSUM") as pap, \
                 tc.tile_pool(name="phAo", bufs=2, space="PSUM") as pao:
                agin_sb = pa.tile([128, c.NBLK, c.ZF], FP, bufs=1)
                adst_sb = pa.tile([128, c.NBLK, c.ZF], FP, bufs=1)
                for ch in range(c.NBLK):
                    vt_ps = pap.tile([c.E, 128], FP, name="vtps")
                    nc.tensor.transpose(vt_ps[:], v_sb[:, ch, :], identF[:])
                    vt = pa.tile([c.E, 128], FP, name="vt")
                    nc.scalar.copy(vt[:], vt_ps[:])
                    a1 = pao.tile([128, c.ZF], FP, name="a1")
                    nc.tensor.matmul(a1[:], vt[:], Wsrc2_sb[:, l * c.ZF:(l + 1) * c.ZF],
                                     start=True, stop=True)
                    nc.scalar.copy(agin_sb[:, ch, :], a1[:])
                    a2 = pao.tile([128, c.ZF], FP, name="a2")
                    nc.tensor.matmul(a2[:], vt[:], Wdst2_sb[:, l * c.ZF:(l + 1) * c.ZF],
                                     start=True, stop=True)
                    nc.vector.tensor_copy(adst_sb[:, ch, :], a2[:])
                nc.sync.dma_start(
                    agin_l[l][:].rearrange("(b p) f -> p b f", p=128), agin_sb[:])
                nc.sync.dma_start(
                    adst_dram[:].rearrange("(b p) f -> p b f", p=128), adst_sb[:])
            nc.gpsimd.collective_compute(
                "AllGather", OP.bypass, replica_groups=RG,
                ins=[agin_l[l].opt()], outs=[agout_l[l].opt()])

            # ---- pass 1: z + stats
            with tc.tile_pool(name="p1idx", bufs=2) as pidx, \
                 tc.tile_pool(name="p1g", bufs=3) as pg, \
                 tc.tile_pool(name="p1z", bufs=2) as pz, \
                 tc.tile_pool(name="p1acc", bufs=1) as pacc, \
                 tc.tile_pool(name="p1zp", bufs=4, space="PSUM") as pzp:
                acc_z = pacc.tile([128, c.GS, c.ZF], FP)
                acc_q = pacc.tile([128, c.GS, c.ZF], FP)
                nc.vector.memset(acc_z[:], 0.0)
                nc.vector.memset(acc_q[:], 0.0)
                for b in (0, 1):
                    nchb = EP[b] // 128
                    base_ch = (0 if b == 0 else EP[0] // 128)
                    for g0 in range(0, nchb, c.GS):
                        gs = min(c.GS, nchb - g0)
                        ni = gs * 128
                        coff = base_ch + g0
                        idxs_t = pidx.tile([128, c.GS * 8], mybir.dt.int16, name="idxs")
                        nc.sync.dma_start(idxs_t[:, :gs * 8],
                                          t_srcrel[:, coff * 8:coff * 8 + gs * 8])
                        idxd_t = pidx.tile([128, c.GS * 8], mybir.dt.int16, name="idxd")
                        nc.sync.dma_start(idxd_t[:, :gs * 8],
                                          t_dstrel[:, coff * 8:coff * 8 + gs * 8])
                        gsr_t = pg.tile([128, c.GS, c.ZF], FP, name="gsrc")
                        nc.gpsimd.dma_gather(
                            gsr_t[:, :gs, :],
                            agout_l[l][b * c.HALF:(b + 1) * c.HALF, :],
                            idxs_t[:, :gs * 8], num_idxs=ni, num_idxs_reg=ni,
                            elem_size=c.ZF, queue_num=gq % 4, single_packet=False)
                        gq += 1
                        gds_t = pg.tile([128, c.GS, c.ZF], FP, name="gdst")
                        nc.gpsimd.dma_gather(
                            gds_t[:, :gs, :],
                            adst_dram[:],
                            idxd_t[:, :gs * 8], num_idxs=ni, num_idxs_reg=ni,
                            elem_size=c.ZF, queue_num=gq % 4, single_packet=False)
                        gq += 1
                        ef_t = pg.tile([c.FE, c.GS * 128], FP, name="eft")
                        nc.sync.dma_start(ef_t[:, :ni],
                                          t_eft[:, coff * 128:coff * 128 + ni])
                        pef_t = pz.tile([128, c.GS, c.ZF], FP, name="peft")
                        for j in range(gs):
                            ps = pzp.tile([128, c.ZF], FP, name="psz")
                            nc.tensor.matmul(ps[:], ef_t[:, j * 128:(j + 1) * 128],
                                             Wef2_sb[:, l * c.ZF:(l + 1) * c.ZF],
                                             start=True, stop=True)
                            nc.scalar.copy(pef_t[:, j, :], ps[:])
                        z_t = pz.tile([128, c.GS, c.ZF], FP, name="zt")
                        nc.any.tensor_add(z_t[:, :gs, :], gsr_t[:, :gs, :], pef_t[:, :gs, :])
                        nc.any.tensor_add(z_t[:, :gs, :], z_t[:, :gs, :], gds_t[:, :gs, :])
                        nc.sync.dma_start(zbuf[:, coff:coff + gs, :], z_t[:, :gs, :])
                        sq_t = pz.tile([128, c.GS, c.ZF], FP, name="sqt")
                        nc.any.tensor_mul(sq_t[:, :gs, :], z_t[:, :gs, :], z_t[:, :gs, :])
                        nc.any.tensor_add(acc_z[:, :gs, :], acc_z[:, :gs, :], z_t[:, :gs, :])
                        nc.any.tensor_add(acc_q[:, :gs, :], acc_q[:, :gs, :], sq_t[:, :gs, :])
                with tc.tile_pool(name="p1st", bufs=1) as pst, \
                     tc.tile_pool(name="p1sp", bufs=1, space="PSUM") as psp:
                    red_z = pst.tile([128, c.ZF], FP)
                    red_q = pst.tile([128, c.ZF], FP)
                    nc.vector.tensor_reduce(
                        red_z[:], acc_z[:].rearrange("p g f -> p f g"),
                        mybir.AxisListType.X, OP.add)
                    nc.vector.tensor_reduce(
                        red_q[:], acc_q[:].rearrange("p g f -> p f g"),
                        mybir.AxisListType.X, OP.add)
                    pss = psp.tile([1, c.ZF], FP, name="pss")
                    psq = psp.tile([1, c.ZF], FP, name="psq")
                    nc.tensor.matmul(pss[:], ones_col[:, :], red_z[:], start=True, stop=True)
                    nc.tensor.matmul(psq[:], ones_col[:, :], red_q[:], start=True, stop=True)
                    stat = pst.tile([1, 2 * c.ZF], FP)
                    nc.vector.tensor_copy(stat[:, :c.ZF], pss[:])
                    nc.vector.tensor_copy(stat[:, c.ZF:], psq[:])
                    nc.sync.dma_start(est_in[l][:], stat[:])

            nc.gpsimd.collective_compute(
                "AllReduce", OP.add, replica_groups=RG,
                ins=[est_in[l].opt()], outs=[est_out[l].opt()])

            # ---- pass 2: activations + scatter
            with tc.tile_pool(name="p2", bufs=1) as p2, \
                 tc.tile_pool(name="p2z", bufs=2) as p2z, \
                 tc.tile_pool(name="p2w", bufs=3) as p2w, \
                 tc.tile_pool(name="p2ap", bufs=4, space="PSUM") as p2ap, \
                 tc.tile_pool(name="p2bp", bufs=1, space="PSUM") as p2bp:
                rstat = p2.tile([1, 2 * c.ZF], FP)
                nc.sync.dma_start(rstat[:], est_out[l][:])
                gms = p2.tile([1, 2 * c.E], FP)
                nc.sync.dma_start(gms[:, :c.E], t_gm[l:l + 1, :])
                nc.sync.dma_start(gms[:, c.E:], t_gs[l:l + 1, :])
                bms = p2.tile([1, 2 * c.E], FP)
                nc.sync.dma_start(bms[:, :c.E], t_bem[l:l + 1, :])
                nc.sync.dma_start(bms[:, c.E:], t_bes[l:l + 1, :])
                st = bn_fold(p2, rstat, c.ZF, c.M, gms[:], bms[:])
                stb = bcast_row(p2, p2bp, st[:], 2 * c.ZF, "edgest")
                s_g = p2.tile([128, c.GS, c.ZF], FP)
                t_g = p2.tile([128, c.GS, c.ZF], FP)
                for j in range(c.GS):
                    nc.vector.tensor_copy(s_g[:, j, :], stb[:, 0:c.ZF])
                    nc.vector.tensor_copy(t_g[:, j, :], stb[:, c.ZF:])
                nc.vector.memset(agg_sb[:], 0.0)
                for b in (0, 1):
                    nchb = EP[b] // 128
                    base_ch = (0 if b == 0 else EP[0] // 128)
                    for g0 in range(0, nchb, c.GS):
                        gs = min(c.GS, nchb - g0)
                        coff = base_ch + g0
                        z_t = p2z.tile([128, c.GS, c.ZF], FP, name="z2t")
                        nc.sync.dma_start(z_t[:, :gs, :], zbuf[:, coff:coff + gs, :])
                        ind_t = p2z.tile([128, c.GS, 128], FP, name="indt")
                        nc.sync.dma_start(
                            ind_t[:, :gs, :],
                            t_indt[coff:coff + gs].rearrange("c p d -> p c d"))
                        u = p2z.tile([128, c.GS, c.ZF], FP, name="u")
                        nc.any.tensor_mul(u[:, :gs, :], z_t[:, :gs, :], s_g[:, :gs, :])
                        nc.any.tensor_add(u[:, :gs, :], u[:, :gs, :], t_g[:, :gs, :])
                        um = u[:, :gs, 0:c.E]
                        us = u[:, :gs, c.E:]
                        # core = ln(1 + exp(-|u|)) on both halves at once
                        core = p2z.tile([128, c.GS, c.ZF], FP, name="core")
                        nc.scalar.activation(core[:, :gs, :], u[:, :gs, :], AF.Abs)
                        nc.scalar.activation(core[:, :gs, :], core[:, :gs, :], AF.Exp, scale=-1.0)
                        nc.any.tensor_scalar_add(core[:, :gs, :], core[:, :gs, :], 1.0)
                        nc.scalar.activation(core[:, :gs, :], core[:, :gs, :], AF.Ln)
                        # softplus(us) = core_s + relu(us)
                        sp = p2z.tile([128, c.GS, c.E], FP, name="sp")
                        nc.scalar.activation(sp[:, :gs, :], us, AF.Relu)
                        nc.any.tensor_add(sp[:, :gs, :], sp[:, :gs, :], core[:, :gs, c.E:])
                        # sigmoid(um) = exp(-(core_m + relu(-um)))
                        sg = p2z.tile([128, c.GS, c.E], FP, name="sg")
                        nc.vector.tensor_scalar(sg[:, :gs, :], um, 0.0, -1.0, OP.min, OP.mult)
                        nc.any.tensor_add(sg[:, :gs, :], sg[:, :gs, :], core[:, :gs, 0:c.E])
                        nc.scalar.activation(sg[:, :gs, :], sg[:, :gs, :], AF.Exp, scale=-1.0)
                        h = p2z.tile([128, c.GS, c.E], FP, name="h")
                        nc.any.tensor_mul(h[:, :gs, :], sg[:, :gs, :], sp[:, :gs, :])
                        for j in range(gs):
                            ch = coff + j
                            ag = p2ap.tile([128, c.E], FP, name="psagg")
                            nc.tensor.matmul(ag[:], ind_t[:, j, :], h[:, j, :],
                                             start=True, stop=True)
                            r = nc.alloc_registers(engines=[DVE])
                            nc.regs_load(r, blkid_sb[0:1, ch:ch + 1])
                            bv = nc.snap(r, donate=True, min_val=0, max_val=c.NBLK - 1)
                            sl = agg_sb[:].rearrange("p b f -> p (b f)")[:, bass.ts(bv, c.E)]
                            nc.vector.tensor_tensor(sl, sl, ag[:], OP.add)

            # ---- node BN + update
            with tc.tile_pool(name="nod", bufs=1) as nod, \
                 tc.tile_pool(name="nodw", bufs=2) as nodw, \
                 tc.tile_pool(name="nodp", bufs=2, space="PSUM") as nodp, \
                 tc.tile_pool(name="nods", bufs=1, space="PSUM") as nods:
                nsum = nods.tile([1, c.E], FP)
                nssq = nods.tile([1, c.E], FP)
                for ch in range(c.NBLK):
                    sq = nodw.tile([128, c.E], FP, name="nsq")
                    nc.vector.tensor_mul(sq[:], agg_sb[:, ch, :], agg_sb[:, ch, :])
                    nc.tensor.matmul(nsum[:], ones_col[:, :], agg_sb[:, ch, :],
                                     start=(ch == 0), stop=(ch == c.NBLK - 1))
                    nc.tensor.matmul(nssq[:], ones_col[:, :], sq[:],
                                     start=(ch == 0), stop=(ch == c.NBLK - 1))
                stat = nod.tile([1, 2 * c.E], FP)
                nc.vector.tensor_copy(stat[:, :c.E], nsum[:])
                nc.vector.tensor_copy(stat[:, c.E:], nssq[:])
                nc.sync.dma_start(nst_in[l][:], stat[:])
                nc.gpsimd.collective_compute(
                    "AllReduce", OP.add, replica_groups=RG,
                    ins=[nst_in[l].opt()], outs=[nst_out[l].opt()])
                rstat = nod.tile([1, 2 * c.E], FP)
                nc.sync.dma_start(rstat[:], nst_out[l][:])
                gn_sb = nod.tile([1, c.E], FP)
                ben_sb = nod.tile([1, c.E], FP)
                nc.sync.dma_start(gn_sb[:], t_gn[l:l + 1, :])
                nc.sync.dma_start(ben_sb[:], t_ben[l:l + 1, :])
                st = bn_fold(nod, rstat, c.E, c.N, gn_sb[:], ben_sb[:])
                stb = bcast_row(nod, nodp, st[:], 2 * c.E, "nodst")
                for ch in range(c.NBLK):
                    u = nodw.tile([128, c.E], FP, name="nu")
                    nc.vector.tensor_mul(u[:], agg_sb[:, ch, :], stb[:, 0:c.E])
                    nc.vector.tensor_add(u[:], u[:], stb[:, c.E:])
                    nc.vector.tensor_add(u[:], u[:], v_sb[:, ch, :])
                    softplus_ops(nodw, v_sb[:, ch, :], u[:], [128, c.E], "nod")
                zero_vpad()

        # ---------------------------------------------------- readout
        with tc.tile_pool(name="ro", bufs=1) as ro, \
             tc.tile_pool(name="row", bufs=2) as row, \
             tc.tile_pool(name="rop", bufs=1, space="PSUM") as rop, \
             tc.tile_pool(name="ros", bufs=1, space="PSUM") as ros:
            psums = ros.tile([c.E, c.NG], FP)
            pcnt = ros.tile([1, c.NG], FP)
            for ch in range(c.NBLK):
                gind = row.tile([128, c.NG], FP, name="gind")
                nc.vector.tensor_scalar(gind[:], iotaF[:, :c.NG],
                                        gid_sb[:, ch:ch + 1], None, OP.is_equal)
                nc.tensor.matmul(psums[:], v_sb[:, ch, :], gind[:],
                                 start=(ch == 0), stop=(ch == c.NBLK - 1))
                nc.tensor.matmul(pcnt[:], ones_col[:, :], gind[:],
                                 start=(ch == 0), stop=(ch == c.NBLK - 1))
            acc = ro.tile([c.E + 1, c.NG], FP)
            nc.scalar.copy(acc[0:c.E, :], psums[:])
            nc.scalar.copy(acc[c.E:c.E + 1, :], pcnt[:])
            nc.sync.dma_start(ro_in[:], acc[:])
            nc.gpsimd.collective_compute(
                "AllReduce", OP.add, replica_groups=RG,
                ins=[ro_in.opt()], outs=[ro_out.opt()])
            racc = ro.tile([c.E + 1, c.NG], FP)
            nc.sync.dma_start(racc[:], ro_out[:])
            cnt = ro.tile([1, c.NG], FP)
            nc.vector.tensor_scalar_max(cnt[:], racc[c.E:c.E + 1, :], 1.0)
            nc.vector.reciprocal(cnt[:], cnt[:])
            rcb_ps = rop.tile([c.E, c.NG], FP, name="rcb", tag="rosc")
            nc.tensor.matmul(rcb_ps[:], ones_row[:, 0:c.E], cnt[:], start=True, stop=True)
            vs = ro.tile([c.E, c.NG], FP)
            nc.vector.tensor_tensor(vs[:], racc[0:c.E, :], rcb_ps[:], OP.mult)

            def fc_bn_silu(pool, psum_pool, x_sb, W_ap, K, Fo, g_t, be_t, nm):
                ps = psum_pool.tile([Fo, c.NG], FP, name=f"fc{nm}", tag="rosc")
                W_sb = pool.tile([K, Fo], FP, name=f"W{nm}")
                nc.sync.dma_start(W_sb[:], W_ap)
                nc.tensor.matmul(ps[:], W_sb[:], x_sb[:], start=True, stop=True)
                g_sb = pool.tile([Fo, 1], FP, name=f"g{nm}")
                be_sb = pool.tile([Fo, 1], FP, name=f"be{nm}")
                nc.sync.dma_start(g_sb[:], g_t[:])
                nc.sync.dma_start(be_sb[:], be_t[:])
                x_sbc = pool.tile([Fo, c.NG], FP, name=f"x{nm}")
                nc.scalar.copy(x_sbc[:], ps[:])
                sums = pool.tile([Fo, 1], FP, name=f"su{nm}")
                nc.vector.tensor_reduce(sums[:], x_sbc[:], mybir.AxisListType.X, OP.add)
                sq = pool.tile([Fo, c.NG], FP, name=f"sq{nm}")
                nc.vector.tensor_mul(sq[:], x_sbc[:], x_sbc[:])
                ssq = pool.tile([Fo, 1], FP, name=f"sl{nm}")
                nc.vector.tensor_reduce(ssq[:], sq[:], mybir.AxisListType.X, OP.add)
                mean = pool.tile([Fo, 1], FP, name=f"mn{nm}")
                nc.scalar.mul(mean[:], sums[:], 1.0 / c.NG)
                var = pool.tile([Fo, 1], FP, name=f"vr{nm}")
                nc.scalar.mul(var[:], ssq[:], 1.0 / c.NG)
                m2 = pool.tile([Fo, 1], FP, name=f"m2{nm}")
                nc.vector.tensor_mul(m2[:], mean[:], mean[:])
                nc.vector.tensor_sub(var[:], var[:], m2[:])
                nc.scalar.activation(var[:], var[:], AF.Ln, bias=epsC[0:Fo, :])
                nc.scalar.activation(var[:], var[:], AF.Exp, scale=-0.5)
                s_col = pool.tile([Fo, 1], FP, name=f"sc{nm}")
                nc.vector.tensor_mul(s_col[:], g_sb[:], var[:])
                t_col = pool.tile([Fo, 1], FP, name=f"tc{nm}")
                nc.vector.tensor_mul(t_col[:], mean[:], s_col[:])
                nc.vector.tensor_sub(t_col[:], be_sb[:], t_col[:])
                u = pool.tile([Fo, c.NG], FP, name=f"u{nm}")
                nc.scalar.activation(u[:], x_sbc[:], AF.Identity,
                                     bias=t_col[:], scale=s_col[:])
                sg2 = pool.tile([Fo, c.NG], FP, name=f"sg{nm}")
                sigmoid_ops(pool, sg2[:], u[:], [Fo, c.NG], f"fc{nm}")
                out = pool.tile([Fo, c.NG], FP, name=f"o{nm}")
                nc.vector.tensor_mul(out[:], u[:], sg2[:])
                return out

            z1 = fc_bn_silu(ro, rop, vs, t_Wf0[:], c.E, c.FC0, t_gf0, t_bef0, "0")
            z2 = fc_bn_silu(ro, rop, z1, t_Wf1[:], c.FC0, c.FC1, t_gf1, t_bef1, "1")
            Wt_sb = ro.tile([c.E, 1], FP)
            nc.sync.dma_start(Wt_sb[:], t_Wt[:])
            hd = rop.tile([1, c.NG], FP, name="hd", tag="rosc")
            nc.tensor.matmul(hd[:], Wt_sb[:], z2[:], start=True, stop=True)
            bt_sb = ro.tile([1, 1], FP)
            nc.sync.dma_start(bt_sb[:], t_bt[:])
            res = ro.tile([1, c.NG], FP)
            nc.vector.tensor_scalar(res[:], hd[:], bt_sb[0:1, 0:1], None, OP.add)
            nc.sync.dma_start(t_out[:], res[:])

    nc.compile()
    return nc


# ------------------------------------------------------------------ driver
_CACHE = {}


def kernel(**inputs):
    cfg = Cfg(int(inputs["node_feats"].shape[0]),
              int(inputs["src"].shape[0]), 256)
    in_maps, EP = preprocess(inputs, cfg)
    key = (cfg.N, cfg.M, tuple(EP))
    if key not in _CACHE:
        _CACHE[key] = build(cfg, EP)
    nc = _CACHE[key]
    res = bass_utils.run_bass_kernel_spmd(
        nc, in_maps, core_ids=list(range(cfg.NC)), trace=False)
    out = np.asarray(res.results[0]["out"], np.float32)
    return out.reshape(cfg.NG, 1)



# revision 12
# speedup vs baseline: 1.1187x; 1.1187x over previous
"""CGCNN (gnn_message_passing) Trainium2 kernel — 8-core SPMD, v2.

Strategy (v2, redesigned from the 7.9ms baseline traced as DMA-bound):
  - Nodes partitioned contiguously across 8 cores (6250/core, padded to 6272);
    edges assigned to the core owning their dst node, sorted by dst, grouped
    into 128-edge chunks that never cross a 128-node dst block.
  - Per conv layer each core computes f16 projection tables
      A_src = v @ [Wm_src|Ws_src]  (AllGathered, f16: half the bytes)
      A_dst = v @ [Wm_dst|Ws_dst]  (local DRAM, f16)
    and gathers them per edge via SWDGE (256B descriptors).
  - z[e] = A_src[src] + A_dst[dst] + ef[e] @ Wef (bf16 matmul into PSUM,
    drained to SBUF by the Act engine; two f16 DVE adds).  z spilled f16.
  - BN stats without big accumulators: sum(z) is computed EXACTLY from
    host-precomputed global degree weights dotted with the local A_src/A_dst
    shards (PE matmuls); sum(z^2) comes from a layer-wide Gram accumulation
    z^T z in PSUM whose diagonal is extracted once per layer. Tiny AllReduce.
  - Pass 2 reloads z, applies the folded BN affine (two f16 DVE ops) and
    sigmoid*softplus built from one act table:
      e = exp(u); sigma = recip(1+e_m) (f16); sp = ln(e_s + 1) (bias fold).
    Scatter-sum via per-chunk indicator matmuls generated ON-CHIP
    (iota==dstblk compare), accumulating straight into a PSUM-resident
    agg[128, NBLK, 64] region through register-offset matmul outputs.
  - Node BN: local sums + tiny AllReduce.  Readout replicated per core.
"""

import sys
import os
from contextlib import ExitStack

sys.path.insert(0, "/opt/trn_rl_repo")

import numpy as np

import concourse.bass as bass
import concourse.bacc as bacc
import concourse.tile as tile
from concourse import mybir, bass_utils
import concourse.hw_specs as hw_specs

FP = mybir.dt.float32
F16 = mybir.dt.float16
BF16 = mybir.dt.bfloat16

# Restrict activation-table selection to one set so the scalar engine never
# reloads tables (everything is built from Exp/Ln/Relu/Identity/Copy).
_KEEP_TABLES = {"natural_log_exp_and_others"}


def _patched_tables(arch):
    t = hw_specs.get_activation_tables(arch)
    return {k: (v if k in _KEEP_TABLES else set()) for k, v in t.items()}


bacc.get_activation_tables = _patched_tables


# ---------------------------------------------------------------- config
class Cfg:
    def __init__(self, N, M, NG):
        self.NC = 8
        self.N, self.M, self.NG = N, M, NG
        self.FV, self.FE, self.E, self.L = 92, 41, 64, 3
        self.FC0, self.FC1 = 128, 64
        self.ZF = 128                       # z width = 2*E
        self.NB = N // self.NC              # real nodes per core
        self.NBP = -(-(self.NB + 1) // 128) * 128  # padded (>= NB+1: zero row)
        self.NBLK = self.NBP // 128
        self.NT = self.NBP * self.NC
        self.HALF = self.NT // 2
        assert self.HALF - 1 < 32768
        assert self.NBP > self.NB
        self.GS = 16                        # chunks per group (2048 edges)
        self.EPS = 1e-5


# ---------------------------------------------------------- preprocessing
def _wrap_idx16(idx):
    a = idx.reshape(-1, 16).T.astype(np.int16)
    return np.tile(a, (8, 1))


def preprocess(inputs, cfg):
    c = cfg
    src = np.asarray(inputs["src"]).astype(np.int64)
    dst = np.asarray(inputs["dst"]).astype(np.int64)
    ef = np.asarray(inputs["edge_feats"], np.float32)
    nf = np.asarray(inputs["node_feats"], np.float32)
    gid = np.asarray(inputs["graph_ids"]).astype(np.int64)

    pad_row = (src // c.NB) * c.NBP + (src % c.NB)
    owner = dst // c.NB
    dst_loc = dst - owner * c.NB

    # global degree weights (counts over ALL edges)
    outdeg = np.bincount(src, minlength=c.N).astype(np.float32)
    indeg = np.bincount(dst, minlength=c.N).astype(np.float32)

    cores = []
    for core in range(c.NC):
        em = np.nonzero(owner == core)[0]
        bucket = (pad_row[em] >= c.HALF).astype(np.int64)
        per_bucket = []
        for b in (0, 1):
            eb = em[bucket == b]
            eb = eb[np.argsort(dst_loc[eb], kind="stable")]
            blk = dst_loc[eb] // 128
            segs = []
            for bk in range(c.NBLK):
                run = eb[blk == bk]
                segs.append((run, bk, (-len(run)) % 128))
            per_bucket.append(segs)
        cores.append(per_bucket)

    EP = [0, 0]
    for b in (0, 1):
        for core in range(c.NC):
            tot = sum(len(r) + p for r, _, p in cores[core][b])
            EP[b] = max(EP[b], tot)
        EP[b] = max(-(-EP[b] // (128 * c.GS)) * (128 * c.GS), 128 * c.GS)
    EPT = EP[0] + EP[1]
    ZROW = c.NB  # all-zero table row (first pad node), same rel id both halves

    Wm = np.asarray(inputs["Wm"], np.float32)
    Ws = np.asarray(inputs["Ws"], np.float32)
    E = c.E
    Wef2 = np.concatenate([Wm[:, 2 * E:, :], Ws[:, 2 * E:, :]], axis=2)  # [L,41,128]

    in_maps = []
    for core in range(c.NC):
        srcrel = np.full(EPT, ZROW, np.int64)
        dstrel = np.full(EPT, ZROW, np.int64)
        dstblk = np.full(EPT, -1.0, np.float32)
        blkid = np.zeros(EPT // 128, np.int32)
        eperm = np.full(EPT, -1, np.int64)
        for b in (0, 1):
            boff = b * EP[0]
            pos = 0
            for run, bk, npad in cores[core][b]:
                n = len(run)
                if n:
                    sl = slice(boff + pos, boff + pos + n)
                    srcrel[sl] = pad_row[run] - b * c.HALF
                    dstrel[sl] = dst_loc[run]
                    dstblk[sl] = (dst_loc[run] - bk * 128).astype(np.float32)
                    eperm[sl] = run
                blkid[(boff + pos) // 128: (boff + pos + n + npad) // 128] = bk
                pos += n + npad

        eft = np.zeros((c.FE, EPT), np.float32)
        real = eperm >= 0
        eft[:, real] = ef[eperm[real]].T

        nfT = np.zeros((c.FV, c.NBP), np.float32)
        nfT[:, : c.NB] = nf[core * c.NB: (core + 1) * c.NB].T
        gidc = np.full(c.NBP, -1.0, np.float32)
        gidc[: c.NB] = gid[core * c.NB: (core + 1) * c.NB].astype(np.float32)

        # degree-weight columns for the exact sum(z) decomposition
        wsrc = np.zeros(c.NBP, np.float32)
        wsrc[: c.NB] = outdeg[core * c.NB: (core + 1) * c.NB]
        wdst = np.zeros(c.NBP, np.float32)
        wdst[: c.NB] = indeg[core * c.NB: (core + 1) * c.NB]

        # per-core edge-feature projection sums: (sum_e ef[e]) @ Wef2[l]
        efsum = ef[eperm[real]].sum(axis=0)  # [41]
        efW = np.stack([efsum @ Wef2[l] for l in range(c.L)], axis=0)  # [L,128]

        m = {
            "srcrel": _wrap_idx16(srcrel.astype(np.int16)),
            "dstrel": _wrap_idx16(dstrel.astype(np.int16)),
            "dstblk": dstblk.reshape(-1, 128).T.copy(),
            "blkid": blkid.reshape(1, -1),
            "eft": eft.astype(np.bfloat16) if hasattr(np, "bfloat16") else eft,
            "nfT": nfT,
            "gidc": gidc.reshape(-1, 128).T.copy(),
            "wsrc": wsrc.reshape(-1, 128).T.astype(np.float16).copy(),
            "wdst": wdst.reshape(-1, 128).T.astype(np.float16).copy(),
            "efW": efW.reshape(1, -1).astype(np.float32),
        }
        in_maps.append(m)

    def to_bf16(x):
        import ml_dtypes
        return x.astype(ml_dtypes.bfloat16)

    shared = {
        "W_emb": np.asarray(inputs["W_emb"], np.float32),
        "g_emb": np.asarray(inputs["g_emb"], np.float32).reshape(1, E),
        "be_emb": np.asarray(inputs["be_emb"], np.float32).reshape(1, E),
        "Wsrc2": to_bf16(np.concatenate([Wm[:, :E, :], Ws[:, :E, :]], axis=2)),
        "Wdst2": to_bf16(np.concatenate([Wm[:, E:2 * E, :], Ws[:, E:2 * E, :]], axis=2)),
        "Wef2": to_bf16(Wef2),
        "gm": np.asarray(inputs["gm"], np.float32),
        "bem": np.asarray(inputs["bem"], np.float32),
        "gs": np.asarray(inputs["gs"], np.float32),
        "bes": np.asarray(inputs["bes"], np.float32),
        "gn": np.asarray(inputs["gn"], np.float32),
        "ben": np.asarray(inputs["ben"], np.float32),
        "Wf0": np.asarray(inputs["Wf0"], np.float32),
        "gf0": np.asarray(inputs["gf0"], np.float32).reshape(-1, 1),
        "bef0": np.asarray(inputs["bef0"], np.float32).reshape(-1, 1),
        "Wf1": np.asarray(inputs["Wf1"], np.float32),
        "gf1": np.asarray(inputs["gf1"], np.float32).reshape(-1, 1),
        "bef1": np.asarray(inputs["bef1"], np.float32).reshape(-1, 1),
        "Wt": np.asarray(inputs["Wt"], np.float32),
        "bt": np.asarray(inputs["bt"], np.float32).reshape(1, 1),
    }
    for m in in_maps:
        # eft conversion (numpy lacks bfloat16; use ml_dtypes)
        m["eft"] = to_bf16(np.asarray(m["eft"], np.float32))
        m.update(shared)
    return in_maps, EP


# ------------------------------------------------------------- kernel build
def build(cfg, EP, dbg=False):
    c = cfg
    EPT = EP[0] + EP[1]
    NCH = EPT // 128
    DVE = mybir.EngineType.DVE
    AF = mybir.ActivationFunctionType
    OP = mybir.AluOpType

    nc = bacc.Bacc("TRN2", target_bir_lowering=False, debug=False,
                   enable_asserts=False, num_devices=c.NC, num_swdge_queues=4)

    def din(name, shape, dt=FP):
        return nc.dram_tensor(name, shape, dt, kind="ExternalInput")

    t_srcrel = din("srcrel", [128, EPT // 16], mybir.dt.int16)
    t_dstrel = din("dstrel", [128, EPT // 16], mybir.dt.int16)
    t_dstblk = din("dstblk", [128, NCH], FP)
    t_blkid = din("blkid", [1, NCH], mybir.dt.int32)
    t_eft = din("eft", [c.FE, EPT], BF16)
    t_nfT = din("nfT", [c.FV, c.NBP])
    t_gidc = din("gidc", [128, c.NBLK])
    t_wsrc = din("wsrc", [128, c.NBLK], F16)
    t_wdst = din("wdst", [128, c.NBLK], F16)
    t_efW = din("efW", [1, c.L * c.ZF])
    t_Wemb = din("W_emb", [c.FV, c.E])
    t_gemb = din("g_emb", [1, c.E])
    t_beemb = din("be_emb", [1, c.E])
    t_Wsrc2 = din("Wsrc2", [c.L, c.E, c.ZF], BF16)
    t_Wdst2 = din("Wdst2", [c.L, c.E, c.ZF], BF16)
    t_Wef2 = din("Wef2", [c.L, c.FE, c.ZF], BF16)
    t_gm = din("gm", [c.L, c.E])
    t_bem = din("bem", [c.L, c.E])
    t_gs = din("gs", [c.L, c.E])
    t_bes = din("bes", [c.L, c.E])
    t_gn = din("gn", [c.L, c.E])
    t_ben = din("ben", [c.L, c.E])
    t_Wf0 = din("Wf0", [c.E, c.FC0])
    t_gf0 = din("gf0", [c.FC0, 1])
    t_bef0 = din("bef0", [c.FC0, 1])
    t_Wf1 = din("Wf1", [c.FC0, c.FC1])
    t_gf1 = din("gf1", [c.FC1, 1])
    t_bef1 = din("bef1", [c.FC1, 1])
    t_Wt = din("Wt", [c.E, 1])
    t_bt = din("bt", [1, 1])
    t_out = nc.dram_tensor("out", [1, c.NG], FP, kind="ExternalOutput")
    if dbg:
        t_dv = nc.dram_tensor("dbg_v", [128, c.NBP // 128 * c.E], FP, kind="ExternalOutput")
        t_dst0 = nc.dram_tensor("dbg_est0", [1, 2 * c.ZF], FP, kind="ExternalOutput")
        t_dagg = nc.dram_tensor("dbg_agg", [128, c.NBP // 128 * c.E], FP, kind="ExternalOutput")
        t_dvl = [nc.dram_tensor(f"dbg_vl{i}", [128, c.NBP // 128 * c.E], FP, kind="ExternalOutput")
                 for i in range(3)]
        t_dnst = [nc.dram_tensor(f"dbg_nst{i}", [1, 2 * c.E], FP, kind="ExternalOutput")
                  for i in range(3)]
        t_dz = nc.dram_tensor("dbg_z", [128, 4 * c.ZF], FP, kind="ExternalOutput")

    RG = [list(range(c.NC))]

    with tile.TileContext(nc) as tc, ExitStack() as es:
        dram = es.enter_context(tc.tile_pool(name="dram", bufs=1, space="DRAM"))
        zbuf = dram.tile([128, NCH, c.ZF], F16)
        adst_dram = dram.tile([c.NBP, c.ZF], F16)
        est_in = [dram.tile([1, 2 * c.ZF], FP, name=f"est_in{i}") for i in range(c.L)]
        est_out = [dram.tile([1, 2 * c.ZF], FP, addr_space="Shared", name=f"est_out{i}")
                   for i in range(c.L)]
        nst_in = [dram.tile([1, 2 * c.E], FP, name=f"nst_in{i}") for i in range(c.L + 1)]
        nst_out = [dram.tile([1, 2 * c.E], FP, addr_space="Shared", name=f"nst_out{i}")
                   for i in range(c.L + 1)]
        agin_l = [dram.tile([c.NBP, c.ZF], F16, name=f"agin{i}") for i in range(c.L)]
        agout_l = [dram.tile([c.NT, c.ZF], F16, addr_space="Shared", name=f"agout{i}")
                   for i in range(c.L)]
        ro_in = dram.tile([c.E + 1, c.NG], FP)
        ro_out = dram.tile([c.E + 1, c.NG], FP, addr_space="Shared")

        konst = es.enter_context(tc.tile_pool(name="konst", bufs=1))
        iotaF = konst.tile([128, 256], FP)
        iota16 = konst.tile([128, 128], F16)
        identF = konst.tile([128, 128], FP)
        ones_row = konst.tile([1, 128], FP)
        ones_col = konst.tile([128, 1], FP)
        epsT = konst.tile([1, 1], FP)
        epsC = konst.tile([128, 1], FP)
        padmask = konst.tile([128, 1], FP)
        with tc.tile_pool(name="ksetup", bufs=1) as ks:
            ii = ks.tile([128, 256], mybir.dt.int32)
            nc.gpsimd.iota(ii[:], pattern=[[1, 256]], base=0, channel_multiplier=0)
            nc.vector.tensor_copy(iotaF[:], ii[:])
            nc.vector.tensor_copy(iota16[:], ii[:, :128])
            ip = ks.tile([128, 1], mybir.dt.int32)
            nc.gpsimd.iota(ip[:], pattern=[[1, 1]], base=0, channel_multiplier=1)
            ipf = ks.tile([128, 1], FP)
            nc.vector.tensor_copy(ipf[:], ip[:])
            nc.vector.tensor_scalar(identF[:], iotaF[:, :128], ipf[:], None, OP.is_equal)
            nc.vector.tensor_scalar(padmask[:], ipf[:], float(c.NB % 128), None, OP.is_lt)
        nc.vector.memset(ones_row[:], 1.0)
        nc.vector.memset(ones_col[:], 1.0)
        nc.vector.memset(epsT[:], c.EPS)
        nc.vector.memset(epsC[:], c.EPS)

        state = es.enter_context(tc.tile_pool(name="state", bufs=1))
        v_sb = state.tile([128, c.NBLK, c.E], FP)
        agg_sb = state.tile([128, c.NBLK, c.E], FP)
        blkid_sb = state.tile([1, NCH], mybir.dt.int32)
        dstblk_sb = state.tile([128, NCH], FP)
        gid_sb = state.tile([128, c.NBLK], FP)
        wsrc_sb = state.tile([128, c.NBLK], F16)
        wdst_sb = state.tile([128, c.NBLK], F16)
        efW_sb = state.tile([1, c.L * c.ZF], FP)
        sz_sb = state.tile([1, c.ZF], FP)
        s_g = state.tile([128, c.GS, c.ZF], F16)
        t_g = state.tile([128, c.GS, c.ZF], F16)
        nc.sync.dma_start(blkid_sb[:], t_blkid[:])
        nc.sync.dma_start(dstblk_sb[:], t_dstblk[:])
        nc.sync.dma_start(gid_sb[:], t_gidc[:])
        nc.sync.dma_start(wsrc_sb[:], t_wsrc[:])
        nc.sync.dma_start(wdst_sb[:], t_wdst[:])
        nc.sync.dma_start(efW_sb[:], t_efW[:])

        wts = es.enter_context(tc.tile_pool(name="wts", bufs=1))
        Wsrc2_sb = wts.tile([c.E, c.L * c.ZF], BF16)
        Wdst2_sb = wts.tile([c.E, c.L * c.ZF], BF16)
        Wef2_sb = wts.tile([c.FE, c.L * c.ZF], BF16)
        for l in range(c.L):
            nc.sync.dma_start(Wsrc2_sb[:, l * c.ZF:(l + 1) * c.ZF], t_Wsrc2[l])
            nc.sync.dma_start(Wdst2_sb[:, l * c.ZF:(l + 1) * c.ZF], t_Wdst2[l])
            nc.sync.dma_start(Wef2_sb[:, l * c.ZF:(l + 1) * c.ZF], t_Wef2[l])

        def bn_fold(pool, sums, F, count, g_ap, be_ap):
            st = pool.tile([1, 2 * F], FP, name=f"bnf{nc.next_id()}")
            mean = pool.tile([1, F], FP, name=f"bnm{nc.next_id()}")
            var = pool.tile([1, F], FP, name=f"bnv{nc.next_id()}")
            nc.scalar.mul(mean[:], sums[:, 0:F], 1.0 / count)
            nc.scalar.mul(var[:], sums[:, F:2 * F], 1.0 / count)
            m2 = pool.tile([1, F], FP, name=f"bn2{nc.next_id()}")
            nc.vector.tensor_mul(m2[:], mean[:], mean[:])
            nc.vector.tensor_sub(var[:], var[:], m2[:])
            nc.scalar.activation(var[:], var[:], AF.Ln, bias=epsT[0:1, 0:1])
            nc.scalar.activation(var[:], var[:], AF.Exp, scale=-0.5)
            nc.vector.tensor_mul(st[:, 0:F], g_ap, var[:])
            nc.vector.tensor_mul(mean[:], mean[:], st[:, 0:F])
            nc.vector.tensor_sub(st[:, F:2 * F], be_ap, mean[:])
            return st

        def bcast_row(pool, psum_pool, row_ap, W, name):
            ps = psum_pool.tile([128, W], FP, name=f"ps{name}")
            nc.tensor.matmul(ps[:], ones_row[:, :], row_ap, start=True, stop=True)
            sb = pool.tile([128, W], FP, name=name)
            nc.scalar.copy(sb[:], ps[:])
            return sb

        def zero_vpad():
            cb = c.NB // 128
            nc.vector.tensor_scalar(v_sb[:, cb, :], v_sb[:, cb, :],
                                    padmask[:], None, OP.mult)

        # ---------------------------------------------------- embedding
        with tc.tile_pool(name="emb", bufs=1) as emb, \
             tc.tile_pool(name="embw", bufs=2) as embw, \
             tc.tile_pool(name="embp", bufs=2, space="PSUM") as embp, \
             tc.tile_pool(name="embs", bufs=1, space="PSUM") as embs:
            nfT_sb = emb.tile([c.FV, c.NBP], FP)
            nc.sync.dma_start(nfT_sb[:], t_nfT[:])
            Wemb_sb = emb.tile([c.FV, c.E], FP)
            nc.sync.dma_start(Wemb_sb[:], t_Wemb[:])
            z0 = emb.tile([128, c.NBLK, c.E], FP)
            ssum = embs.tile([1, c.E], FP)
            ssq = embs.tile([1, c.E], FP)
            for ch in range(c.NBLK):
                ps = embp.tile([128, c.E], FP, name="embz")
                nc.tensor.matmul(ps[:], nfT_sb[:, ch * 128:(ch + 1) * 128],
                                 Wemb_sb[:], start=True, stop=True)
                nc.scalar.copy(z0[:, ch, :], ps[:])
                sq = embw.tile([128, c.E], FP, name="embsq")
                nc.vector.tensor_mul(sq[:], z0[:, ch, :], z0[:, ch, :])
                nc.tensor.matmul(ssum[:], ones_col[:, :], z0[:, ch, :],
                                 start=(ch == 0), stop=(ch == c.NBLK - 1))
                nc.tensor.matmul(ssq[:], ones_col[:, :], sq[:],
                                 start=(ch == 0), stop=(ch == c.NBLK - 1))
            stat = emb.tile([1, 2 * c.E], FP)
            nc.vector.tensor_copy(stat[:, 0:c.E], ssum[:])
            nc.vector.tensor_copy(stat[:, c.E:], ssq[:])
            nc.sync.dma_start(nst_in[c.L][:], stat[:])
            nc.gpsimd.collective_compute(
                "AllReduce", OP.add, replica_groups=RG,
                ins=[nst_in[c.L].opt()], outs=[nst_out[c.L].opt()])
            rstat = emb.tile([1, 2 * c.E], FP)
            nc.sync.dma_start(rstat[:], nst_out[c.L][:])
            gemb_sb = emb.tile([1, c.E], FP)
            beemb_sb = emb.tile([1, c.E], FP)
            nc.sync.dma_start(gemb_sb[:], t_gemb[:])
            nc.sync.dma_start(beemb_sb[:], t_beemb[:])
            st = bn_fold(emb, rstat, c.E, c.N, gemb_sb[:], beemb_sb[:])
            stb = bcast_row(emb, embp, st[:], 2 * c.E, "embst")
            for ch in range(c.NBLK):
                u = embw.tile([128, c.E], FP, name="embu")
                nc.vector.tensor_mul(u[:], z0[:, ch, :], stb[:, 0:c.E])
                nc.vector.tensor_add(u[:], u[:], stb[:, c.E:])
                # silu(u) = u / (1 + exp(-u))
                e = embw.tile([128, c.E], FP, name="embe")
                nc.scalar.activation(e[:], u[:], AF.Exp, scale=-1.0)
                nc.vector.tensor_scalar_add(e[:], e[:], 1.0)
                r = embw.tile([128, c.E], FP, name="embr")
                nc.vector.reciprocal(r[:], e[:])
                nc.vector.tensor_mul(v_sb[:, ch, :], u[:], r[:])
            zero_vpad()
        if dbg:
            nc.sync.dma_start(t_dv[:], v_sb[:].rearrange("p b f -> p (b f)"))

        # ---------------------------------------------------- conv layers
        gq = 0
        for l in range(c.L):
            # ---- phase A: projection tables (f16) + exact sum(z) dots
            with tc.tile_pool(name="phA", bufs=2) as pa, \
                 tc.tile_pool(name="phAp", bufs=2, space="PSUM") as pap, \
                 tc.tile_pool(name="phAo", bufs=2, space="PSUM") as pao, \
                 tc.tile_pool(name="phAs", bufs=1, space="PSUM") as pas:
                asrc_sb = pa.tile([128, c.NBLK, c.ZF], F16, bufs=1)
                adst_sb = pa.tile([128, c.NBLK, c.ZF], F16, bufs=1)
                for ch in range(c.NBLK):
                    vt_ps = pap.tile([c.E, 128], FP, name="vtps")
                    nc.tensor.transpose(vt_ps[:], v_sb[:, ch, :], identF[:])
                    vt = pa.tile([c.E, 128], BF16, name="vt")
                    nc.scalar.copy(vt[:], vt_ps[:])
                    a1 = pao.tile([128, c.ZF], FP, name="a1")
                    nc.tensor.matmul(a1[:], vt[:], Wsrc2_sb[:, l * c.ZF:(l + 1) * c.ZF],
                                     start=True, stop=True)
                    nc.scalar.copy(asrc_sb[:, ch, :], a1[:])
                    a2 = pao.tile([128, c.ZF], FP, name="a2")
                    nc.tensor.matmul(a2[:], vt[:], Wdst2_sb[:, l * c.ZF:(l + 1) * c.ZF],
                                     start=True, stop=True)
                    nc.vector.tensor_copy(adst_sb[:, ch, :], a2[:])
                nc.sync.dma_start(
                    agin_l[l][:].rearrange("(b p) f -> p b f", p=128), asrc_sb[:])
                nc.sync.dma_start(
                    adst_dram[:].rearrange("(b p) f -> p b f", p=128), adst_sb[:])
                # exact sum(z): degree-weighted dots over local shards
                szsrc = pas.tile([1, c.ZF], FP)
                szdst = pas.tile([1, c.ZF], FP)
                for ch in range(c.NBLK):
                    nc.tensor.matmul(szsrc[:], wsrc_sb[:, ch:ch + 1],
                                     asrc_sb[:, ch, :],
                                     start=(ch == 0), stop=(ch == c.NBLK - 1))
                    nc.tensor.matmul(szdst[:], wdst_sb[:, ch:ch + 1],
                                     adst_sb[:, ch, :],
                                     start=(ch == 0), stop=(ch == c.NBLK - 1))
                nc.vector.tensor_copy(sz_sb[:], szsrc[:])
                nc.vector.tensor_add(sz_sb[:], sz_sb[:], szdst[:])
                nc.vector.tensor_add(sz_sb[:], sz_sb[:],
                                     efW_sb[:, l * c.ZF:(l + 1) * c.ZF])
            nc.gpsimd.collective_compute(
                "AllGather", OP.bypass, replica_groups=RG,
                ins=[agin_l[l].opt()], outs=[agout_l[l].opt()])

            # ---- pass 1: z assembly + f16 spill + Gram stats
            with tc.tile_pool(name="p1idx", bufs=2) as pidx, \
                 tc.tile_pool(name="p1g", bufs=3) as pg, \
                 tc.tile_pool(name="p1z", bufs=2) as pz, \
                 tc.tile_pool(name="p1st", bufs=1) as p1st, \
                 tc.tile_pool(name="p1zp", bufs=1, space="PSUM") as pzp, \
                 tc.tile_pool(name="p1gr", bufs=1, space="PSUM") as pgr:
                gram = pgr.tile([128, c.ZF], FP)
                first_mm = [True]
                for b in (0, 1):
                    nchb = EP[b] // 128
                    base_ch = (0 if b == 0 else EP[0] // 128)
                    for g0 in range(0, nchb, c.GS):
                        gs = min(c.GS, nchb - g0)
                        ni = gs * 128
                        coff = base_ch + g0
                        last_grp = (b == 1 and g0 + c.GS >= nchb)
                        idxs_t = pidx.tile([128, c.GS * 8], mybir.dt.int16, name="idxs")
                        nc.sync.dma_start(idxs_t[:, :gs * 8],
                                          t_srcrel[:, coff * 8:coff * 8 + gs * 8])
                        idxd_t = pidx.tile([128, c.GS * 8], mybir.dt.int16, name="idxd")
                        nc.sync.dma_start(idxd_t[:, :gs * 8],
                                          t_dstrel[:, coff * 8:coff * 8 + gs * 8])
                        gsr_t = pg.tile([128, c.GS, c.ZF], F16, name="gsrc")
                        nc.gpsimd.dma_gather(
                            gsr_t[:, :gs, :],
                            agout_l[l][b * c.HALF:(b + 1) * c.HALF, :],
                            idxs_t[:, :gs * 8], num_idxs=ni, num_idxs_reg=ni,
                            elem_size=c.ZF, queue_num=gq % 4, single_packet=False)
                        gq += 1
                        gds_t = pg.tile([128, c.GS, c.ZF], F16, name="gdst")
                        nc.gpsimd.dma_gather(
                            gds_t[:, :gs, :],
                            adst_dram[:],
                            idxd_t[:, :gs * 8], num_idxs=ni, num_idxs_reg=ni,
                            elem_size=c.ZF, queue_num=gq % 4, single_packet=False)
                        gq += 1
                        ef_t = pg.tile([c.FE, c.GS * 128], BF16, name="eft")
                        nc.sync.dma_start(ef_t[:, :ni],
                                          t_eft[:, coff * 128:coff * 128 + ni])
                        zp = pzp.tile([128, c.GS, c.ZF], FP, name="zp")
                        for j in range(gs):
                            nc.tensor.matmul(zp[:, j, :], ef_t[:, j * 128:(j + 1) * 128],
                                             Wef2_sb[:, l * c.ZF:(l + 1) * c.ZF],
                                             start=True, stop=True)
                        pef_t = pz.tile([128, c.GS, c.ZF], F16, name="peft")
                        nc.scalar.copy(pef_t[:, :gs, :], zp[:, :gs, :])
                        t1 = pz.tile([128, c.GS, c.ZF], F16, name="t1")
                        nc.vector.tensor_add(t1[:, :gs, :], gsr_t[:, :gs, :],
                                             gds_t[:, :gs, :])
                        z_t = pz.tile([128, c.GS, c.ZF], F16, name="zt")
                        nc.vector.tensor_add(z_t[:, :gs, :], t1[:, :gs, :],
                                             pef_t[:, :gs, :])
                        nc.sync.dma_start(zbuf[:, coff:coff + gs, :], z_t[:, :gs, :])
                        for j in range(gs):
                            nc.tensor.matmul(gram[:], z_t[:, j, :], z_t[:, j, :],
                                             start=first_mm[0],
                                             stop=(last_grp and j == gs - 1))
                            first_mm[0] = False
                # stats: diag(gram) -> row; assemble [sum(z) | sum(z^2)]
                gI = p1st.tile([128, c.ZF], FP)
                nc.vector.tensor_mul(gI[:], gram[:], identF[:])
                with tc.tile_pool(name="p1sq", bufs=1, space="PSUM") as psq:
                    szq = psq.tile([1, c.ZF], FP)
                    nc.tensor.matmul(szq[:], ones_col[:, :], gI[:], start=True, stop=True)
                    stat = p1st.tile([1, 2 * c.ZF], FP)
                    nc.vector.tensor_copy(stat[:, :c.ZF], sz_sb[:])
                    nc.vector.tensor_copy(stat[:, c.ZF:], szq[:])
                    nc.sync.dma_start(est_in[l][:], stat[:])

            nc.gpsimd.collective_compute(
                "AllReduce", OP.add, replica_groups=RG,
                ins=[est_in[l].opt()], outs=[est_out[l].opt()])

            # ---- fold BN affine, broadcast s/t to [128, GS, ZF] f16
            with tc.tile_pool(name="pmid", bufs=1) as pm, \
                 tc.tile_pool(name="pmidp", bufs=1, space="PSUM") as pmp:
                rstat = pm.tile([1, 2 * c.ZF], FP)
                nc.sync.dma_start(rstat[:], est_out[l][:])
                gms = pm.tile([1, 2 * c.E], FP)
                nc.sync.dma_start(gms[:, :c.E], t_gm[l:l + 1, :])
                nc.sync.dma_start(gms[:, c.E:], t_gs[l:l + 1, :])
                bms = pm.tile([1, 2 * c.E], FP)
                nc.sync.dma_start(bms[:, :c.E], t_bem[l:l + 1, :])
                nc.sync.dma_start(bms[:, c.E:], t_bes[l:l + 1, :])
                if dbg and l == 0:
                    nc.sync.dma_start(t_dst0[:], rstat[:])
                    zdbg = pm.tile([128, 4, c.ZF], F16, name="zdbg")
                    nc.sync.dma_start(zdbg[:], zbuf[:, 0:4, :])
                    zdbgf = pm.tile([128, 4 * c.ZF], FP, name="zdbgf")
                    nc.vector.tensor_copy(zdbgf[:], zdbg[:].rearrange("p b f -> p (b f)"))
                    nc.sync.dma_start(t_dz[:], zdbgf[:])
                st = bn_fold(pm, rstat, c.ZF, c.M, gms[:], bms[:])
                stb = bcast_row(pm, pmp, st[:], 2 * c.ZF, "edgest")
                # negate the sigmoid (m) half so exp(u) = exp(-um) there
                nc.vector.tensor_scalar(stb[:, 0:c.E], stb[:, 0:c.E],
                                        -1.0, None, OP.mult)
                nc.vector.tensor_scalar(stb[:, c.ZF:c.ZF + c.E],
                                        stb[:, c.ZF:c.ZF + c.E],
                                        -1.0, None, OP.mult)
                for j in range(c.GS):
                    nc.vector.tensor_copy(s_g[:, j, :], stb[:, 0:c.ZF])
                    nc.vector.tensor_copy(t_g[:, j, :], stb[:, c.ZF:])

            # ---- pass 2: activations + PSUM-resident scatter
            with tc.tile_pool(name="p2z", bufs=2) as p2z, \
                 tc.tile_pool(name="p2ap", bufs=1, space="PSUM") as p2ap:
                agg_ps = p2ap.tile([128, c.NBLK, c.E], FP)
                agg_flat = agg_ps[:].rearrange("p b f -> p (b f)")
                nc.vector.memset(agg_ps[:], 0.0)
                n_sc = [0]
                total_sc = NCH
                for b in (0, 1):
                    nchb = EP[b] // 128
                    base_ch = (0 if b == 0 else EP[0] // 128)
                    for g0 in range(0, nchb, c.GS):
                        gs = min(c.GS, nchb - g0)
                        coff = base_ch + g0
                        z_t = p2z.tile([128, c.GS, c.ZF], F16, name="z2t")
                        nc.sync.dma_start(z_t[:, :gs, :], zbuf[:, coff:coff + gs, :])
                        u = p2z.tile([128, c.GS, c.ZF], F16, name="u")
                        nc.vector.tensor_mul(u[:, :gs, :], z_t[:, :gs, :], s_g[:, :gs, :])
                        nc.vector.tensor_add(u[:, :gs, :], u[:, :gs, :], t_g[:, :gs, :])
                        uc = p2z.tile([128, c.GS, c.ZF], F16, name="uc")
                        nc.vector.tensor_scalar_min(uc[:, :gs, :], u[:, :gs, :], 11.0)
                        e_t = p2z.tile([128, c.GS, c.ZF], F16, name="et")
                        nc.scalar.activation(e_t[:, :gs, :], uc[:, :gs, :], AF.Exp)
                        e1m = p2z.tile([128, c.GS, c.E], F16, name="e1m")
                        nc.vector.tensor_scalar_add(e1m[:, :gs, :],
                                                    e_t[:, :gs, 0:c.E], 1.0)
                        sg_t = p2z.tile([128, c.GS, c.E], F16, name="sgt")
                        with nc.allow_low_precision("sigmoid in f16 is plenty here"):
                            nc.vector.reciprocal(sg_t[:, :gs, :], e1m[:, :gs, :])
                        sp_t = p2z.tile([128, c.GS, c.E], F16, name="spt")
                        nc.scalar.activation(sp_t[:, :gs, :], e_t[:, :gs, c.E:],
                                             AF.Ln, bias=1.0)
                        # softplus(u) == u at f16 precision for u > 11 (clamped above)
                        nc.vector.tensor_tensor(sp_t[:, :gs, :], sp_t[:, :gs, :],
                                                u[:, :gs, c.E:], OP.max)
                        h = p2z.tile([128, c.GS, c.E], F16, name="h")
                        nc.vector.tensor_mul(h[:, :gs, :], sg_t[:, :gs, :],
                                             sp_t[:, :gs, :])
                        ind_t = p2z.tile([128, c.GS, 128], F16, name="ind")
                        for j in range(gs):
                            ch = coff + j
                            nc.vector.tensor_scalar(ind_t[:, j, :], iota16[:, :],
                                                    dstblk_sb[:, ch:ch + 1], None,
                                                    OP.is_equal)
                            r = nc.alloc_registers(engines=[mybir.EngineType.PE])
                            nc.regs_load(r, blkid_sb[0:1, ch:ch + 1])
                            bv = nc.snap(r, donate=True, min_val=0, max_val=c.NBLK - 1)
                            n_sc[0] += 1
                            nc.tensor.matmul(
                                agg_flat[:, bass.ts(bv, c.E)],
                                ind_t[:, j, :], h[:, j, :],
                                start=False, stop=(n_sc[0] == total_sc),
                                skip_group_check=True)
                # drain agg PSUM -> SBUF
                for o in range(0, c.NBLK * c.E, 512):
                    w = min(512, c.NBLK * c.E - o)
                    nc.scalar.copy(
                        agg_sb[:].rearrange("p b f -> p (b f)")[:, o:o + w],
                        agg_flat[:, o:o + w])

            if dbg and l == 0:
                nc.sync.dma_start(t_dagg[:], agg_sb[:].rearrange("p b f -> p (b f)"))

            # ---- node BN + update
            with tc.tile_pool(name="nod", bufs=1) as nod, \
                 tc.tile_pool(name="nodw", bufs=2) as nodw, \
                 tc.tile_pool(name="nodp", bufs=2, space="PSUM") as nodp, \
                 tc.tile_pool(name="nods", bufs=1, space="PSUM") as nods:
                nsum = nods.tile([1, c.E], FP)
                nssq = nods.tile([1, c.E], FP)
                for ch in range(c.NBLK):
                    sq = nodw.tile([128, c.E], FP, name="nsq")
                    nc.vector.tensor_mul(sq[:], agg_sb[:, ch, :], agg_sb[:, ch, :])
                    nc.tensor.matmul(nsum[:], ones_col[:, :], agg_sb[:, ch, :],
                                     start=(ch == 0), stop=(ch == c.NBLK - 1))
                    nc.tensor.matmul(nssq[:], ones_col[:, :], sq[:],
                                     start=(ch == 0), stop=(ch == c.NBLK - 1))
                stat = nod.tile([1, 2 * c.E], FP)
                nc.vector.tensor_copy(stat[:, :c.E], nsum[:])
                nc.vector.tensor_copy(stat[:, c.E:], nssq[:])
                nc.sync.dma_start(nst_in[l][:], stat[:])
                nc.gpsimd.collective_compute(
                    "AllReduce", OP.add, replica_groups=RG,
                    ins=[nst_in[l].opt()], outs=[nst_out[l].opt()])
                rstat = nod.tile([1, 2 * c.E], FP)
                nc.sync.dma_start(rstat[:], nst_out[l][:])
                gn_sb = nod.tile([1, c.E], FP)
                ben_sb = nod.tile([1, c.E], FP)
                nc.sync.dma_start(gn_sb[:], t_gn[l:l + 1, :])
                nc.sync.dma_start(ben_sb[:], t_ben[l:l + 1, :])
                st = bn_fold(nod, rstat, c.E, c.N, gn_sb[:], ben_sb[:])
                stb = bcast_row(nod, nodp, st[:], 2 * c.E, "nodst")
                for ch in range(c.NBLK):
                    u = nodw.tile([128, c.E], FP, name="nu")
                    nc.vector.tensor_mul(u[:], agg_sb[:, ch, :], stb[:, 0:c.E])
                    nc.vector.tensor_add(u[:], u[:], stb[:, c.E:])
                    nc.vector.tensor_add(u[:], u[:], v_sb[:, ch, :])
                    # softplus(u) = ln(1 + exp(u)); u is comfortably < 80
                    e = nodw.tile([128, c.E], FP, name="ne")
                    nc.scalar.activation(e[:], u[:], AF.Exp)
                    nc.scalar.activation(v_sb[:, ch, :], e[:], AF.Ln, bias=1.0)
                zero_vpad()
                if dbg:
                    nc.sync.dma_start(t_dvl[l][:], v_sb[:].rearrange("p b f -> p (b f)"))
                    nc.sync.dma_start(t_dnst[l][:], rstat[:])

        # ---------------------------------------------------- readout
        with tc.tile_pool(name="ro", bufs=1) as ro, \
             tc.tile_pool(name="row", bufs=2) as row, \
             tc.tile_pool(name="rop", bufs=1, space="PSUM") as rop, \
             tc.tile_pool(name="ros", bufs=1, space="PSUM") as ros:
            psums = ros.tile([c.E, c.NG], FP)
            pcnt = ros.tile([1, c.NG], FP)
            for ch in range(c.NBLK):
                gind = row.tile([128, c.NG], FP, name="gind")
                nc.vector.tensor_scalar(gind[:], iotaF[:, :c.NG],
                                        gid_sb[:, ch:ch + 1], None, OP.is_equal)
                nc.tensor.matmul(psums[:], v_sb[:, ch, :], gind[:],
                                 start=(ch == 0), stop=(ch == c.NBLK - 1))
                nc.tensor.matmul(pcnt[:], ones_col[:, :], gind[:],
                                 start=(ch == 0), stop=(ch == c.NBLK - 1))
            acc = ro.tile([c.E + 1, c.NG], FP)
            nc.scalar.copy(acc[0:c.E, :], psums[:])
            nc.scalar.copy(acc[c.E:c.E + 1, :], pcnt[:])
            nc.sync.dma_start(ro_in[:], acc[:])
            nc.gpsimd.collective_compute(
                "AllReduce", OP.add, replica_groups=RG,
                ins=[ro_in.opt()], outs=[ro_out.opt()])
            racc = ro.tile([c.E + 1, c.NG], FP)
            nc.sync.dma_start(racc[:], ro_out[:])
            cnt = ro.tile([1, c.NG], FP)
            nc.vector.tensor_scalar_max(cnt[:], racc[c.E:c.E + 1, :], 1.0)
            nc.vector.reciprocal(cnt[:], cnt[:])
            rcb_ps = rop.tile([c.E, c.NG], FP, name="rcb", tag="rosc")
            nc.tensor.matmul(rcb_ps[:], ones_row[:, 0:c.E], cnt[:], start=True, stop=True)
            vs = ro.tile([c.E, c.NG], FP)
            nc.vector.tensor_tensor(vs[:], racc[0:c.E, :], rcb_ps[:], OP.mult)

            def fc_bn_silu(pool, psum_pool, x_sb, W_ap, K, Fo, g_t, be_t, nm):
                ps = psum_pool.tile([Fo, c.NG], FP, name=f"fc{nm}", tag="rosc")
                W_sb = pool.tile([K, Fo], FP, name=f"W{nm}")
                nc.sync.dma_start(W_sb[:], W_ap)
                nc.tensor.matmul(ps[:], W_sb[:], x_sb[:], start=True, stop=True)
                g_sb = pool.tile([Fo, 1], FP, name=f"g{nm}")
                be_sb = pool.tile([Fo, 1], FP, name=f"be{nm}")
                nc.sync.dma_start(g_sb[:], g_t[:])
                nc.sync.dma_start(be_sb[:], be_t[:])
                x_sbc = pool.tile([Fo, c.NG], FP, name=f"x{nm}")
                nc.scalar.copy(x_sbc[:], ps[:])
                sums = pool.tile([Fo, 1], FP, name=f"su{nm}")
                nc.vector.tensor_reduce(sums[:], x_sbc[:], mybir.AxisListType.X, OP.add)
                sq = pool.tile([Fo, c.NG], FP, name=f"sq{nm}")
                nc.vector.tensor_mul(sq[:], x_sbc[:], x_sbc[:])
                ssq = pool.tile([Fo, 1], FP, name=f"sl{nm}")
                nc.vector.tensor_reduce(ssq[:], sq[:], mybir.AxisListType.X, OP.add)
                mean = pool.tile([Fo, 1], FP, name=f"mn{nm}")
                nc.scalar.mul(mean[:], sums[:], 1.0 / c.NG)
                var = pool.tile([Fo, 1], FP, name=f"vr{nm}")
                nc.scalar.mul(var[:], ssq[:], 1.0 / c.NG)
                m2 = pool.tile([Fo, 1], FP, name=f"m2{nm}")
                nc.vector.tensor_mul(m2[:], mean[:], mean[:])
                nc.vector.tensor_sub(var[:], var[:], m2[:])
                nc.scalar.activation(var[:], var[:], AF.Ln, bias=epsC[0:Fo, :])
                nc.scalar.activation(var[:], var[:], AF.Exp, scale=-0.5)
                s_col = pool.tile([Fo, 1], FP, name=f"sc{nm}")
                nc.vector.tensor_mul(s_col[:], g_sb[:], var[:])
                t_col = pool.tile([Fo, 1], FP, name=f"tc{nm}")
                nc.vector.tensor_mul(t_col[:], mean[:], s_col[:])
                nc.vector.tensor_sub(t_col[:], be_sb[:], t_col[:])
                u = pool.tile([Fo, c.NG], FP, name=f"u{nm}")
                nc.scalar.activation(u[:], x_sbc[:], AF.Identity,
                                     bias=t_col[:], scale=s_col[:])
                # silu
                e = pool.tile([Fo, c.NG], FP, name=f"e{nm}")
                nc.scalar.activation(e[:], u[:], AF.Exp, scale=-1.0)
                nc.vector.tensor_scalar_add(e[:], e[:], 1.0)
                rr = pool.tile([Fo, c.NG], FP, name=f"r{nm}")
                nc.vector.reciprocal(rr[:], e[:])
                out = pool.tile([Fo, c.NG], FP, name=f"o{nm}")
                nc.vector.tensor_mul(out[:], u[:], rr[:])
                return out

            z1 = fc_bn_silu(ro, rop, vs, t_Wf0[:], c.E, c.FC0, t_gf0, t_bef0, "0")
            z2 = fc_bn_silu(ro, rop, z1, t_Wf1[:], c.FC0, c.FC1, t_gf1, t_bef1, "1")
            Wt_sb = ro.tile([c.E, 1], FP)
            nc.sync.dma_start(Wt_sb[:], t_Wt[:])
            hd = rop.tile([1, c.NG], FP, name="hd", tag="rosc")
            nc.tensor.matmul(hd[:], Wt_sb[:], z2[:], start=True, stop=True)
            bt_sb = ro.tile([1, 1], FP)
            nc.sync.dma_start(bt_sb[:], t_bt[:])
            res = ro.tile([1, c.NG], FP)
            nc.vector.tensor_scalar(res[:], hd[:], bt_sb[0:1, 0:1], None, OP.add)
            nc.sync.dma_start(t_out[:], res[:])

    nc.compile()
    return nc


# ------------------------------------------------------------------ driver
_CACHE = {}


def kernel(**inputs):
    cfg = Cfg(int(inputs["node_feats"].shape[0]),
              int(inputs["src"].shape[0]), 256)
    in_maps, EP = preprocess(inputs, cfg)
    key = (cfg.N, cfg.M, tuple(EP))
    if key not in _CACHE:
        _CACHE[key] = build(cfg, EP)
    nc = _CACHE[key]
    res = bass_utils.run_bass_kernel_spmd(
        nc, in_maps, core_ids=list(range(cfg.NC)), trace=False)
    out = np.asarray(res.results[0]["out"], np.float32)
    return out.reshape(cfg.NG, 1)


# revision 13
# speedup vs baseline: 1.3987x; 1.2503x over previous
"""CGCNN (gnn_message_passing) Trainium2 kernel — 8-core SPMD, v2.

Strategy (v2, redesigned from the 7.9ms baseline traced as DMA-bound):
  - Nodes partitioned contiguously across 8 cores (6250/core, padded to 6272);
    edges assigned to the core owning their dst node, sorted by dst, grouped
    into 128-edge chunks that never cross a 128-node dst block.
  - Per conv layer each core computes f16 projection tables
      A_src = v @ [Wm_src|Ws_src]  (AllGathered, f16: half the bytes)
      A_dst = v @ [Wm_dst|Ws_dst]  (local DRAM, f16)
    and gathers them per edge via SWDGE (256B descriptors).
  - z[e] = A_src[src] + A_dst[dst] + ef[e] @ Wef (bf16 matmul into PSUM,
    drained to SBUF by the Act engine; two f16 DVE adds).  z spilled f16.
  - BN stats without big accumulators: sum(z) is computed EXACTLY from
    host-precomputed global degree weights dotted with the local A_src/A_dst
    shards (PE matmuls); sum(z^2) comes from a layer-wide Gram accumulation
    z^T z in PSUM whose diagonal is extracted once per layer. Tiny AllReduce.
  - Pass 2 reloads z, applies the folded BN affine (two f16 DVE ops) and
    sigmoid*softplus built from one act table:
      e = exp(u); sigma = recip(1+e_m) (f16); sp = ln(e_s + 1) (bias fold).
    Scatter-sum via per-chunk indicator matmuls generated ON-CHIP
    (iota==dstblk compare), accumulating straight into a PSUM-resident
    agg[128, NBLK, 64] region through register-offset matmul outputs.
  - Node BN: local sums + tiny AllReduce.  Readout replicated per core.
"""

import sys
import os
from contextlib import ExitStack

sys.path.insert(0, "/opt/trn_rl_repo")

import numpy as np

import concourse.bass as bass
import concourse.bacc as bacc
import concourse.tile as tile
from concourse import mybir, bass_utils
import concourse.hw_specs as hw_specs

FP = mybir.dt.float32
F16 = mybir.dt.float16
BF16 = mybir.dt.bfloat16

# Restrict activation-table selection to one set so the scalar engine never
# reloads tables (everything is built from Exp/Ln/Relu/Identity/Copy).
_KEEP_TABLES = {"natural_log_exp_and_others"}


def _patched_tables(arch):
    t = hw_specs.get_activation_tables(arch)
    return {k: (v if k in _KEEP_TABLES else set()) for k, v in t.items()}


bacc.get_activation_tables = _patched_tables


# ---------------------------------------------------------------- config
class Cfg:
    def __init__(self, N, M, NG):
        self.NC = 8
        self.N, self.M, self.NG = N, M, NG
        self.FV, self.FE, self.E, self.L = 92, 41, 64, 3
        self.FC0, self.FC1 = 128, 64
        self.ZF = 128                       # z width = 2*E
        self.NB = N // self.NC              # real nodes per core
        self.NBP = -(-(self.NB + 1) // 128) * 128  # padded (>= NB+1: zero row)
        self.NBLK = self.NBP // 128
        self.NT = self.NBP * self.NC
        self.HALF = self.NT // 2
        assert self.HALF - 1 < 32768
        assert self.NBP > self.NB
        self.GS = 16                        # chunks per group (2048 edges)
        self.EPS = 1e-5


# ---------------------------------------------------------- preprocessing
def _wrap_idx16(idx):
    a = idx.reshape(-1, 16).T.astype(np.int16)
    return np.tile(a, (8, 1))


def preprocess(inputs, cfg):
    c = cfg
    src = np.asarray(inputs["src"]).astype(np.int64)
    dst = np.asarray(inputs["dst"]).astype(np.int64)
    ef = np.asarray(inputs["edge_feats"], np.float32)
    nf = np.asarray(inputs["node_feats"], np.float32)
    gid = np.asarray(inputs["graph_ids"]).astype(np.int64)

    pad_row = (src // c.NB) * c.NBP + (src % c.NB)
    owner = dst // c.NB
    dst_loc = dst - owner * c.NB

    # global degree weights (counts over ALL edges)
    outdeg = np.bincount(src, minlength=c.N).astype(np.float32)
    indeg = np.bincount(dst, minlength=c.N).astype(np.float32)

    cores = []
    for core in range(c.NC):
        em = np.nonzero(owner == core)[0]
        bucket = (pad_row[em] >= c.HALF).astype(np.int64)
        per_bucket = []
        for b in (0, 1):
            eb = em[bucket == b]
            eb = eb[np.argsort(dst_loc[eb], kind="stable")]
            blk = dst_loc[eb] // 128
            segs = []
            for bk in range(c.NBLK):
                run = eb[blk == bk]
                segs.append((run, bk, (-len(run)) % 128))
            per_bucket.append(segs)
        cores.append(per_bucket)

    EP = [0, 0]
    for b in (0, 1):
        for core in range(c.NC):
            tot = sum(len(r) + p for r, _, p in cores[core][b])
            EP[b] = max(EP[b], tot)
        EP[b] = max(-(-EP[b] // (128 * c.GS)) * (128 * c.GS), 128 * c.GS)
    EPT = EP[0] + EP[1]
    ZROW = c.NB  # all-zero table row (first pad node), same rel id both halves

    Wm = np.asarray(inputs["Wm"], np.float32)
    Ws = np.asarray(inputs["Ws"], np.float32)
    E = c.E
    Wef2 = np.concatenate([Wm[:, 2 * E:, :], Ws[:, 2 * E:, :]], axis=2)  # [L,41,128]

    in_maps = []
    for core in range(c.NC):
        srcrel = np.full(EPT, ZROW, np.int64)
        dstrel = np.full(EPT, ZROW, np.int64)
        dstblk = np.full(EPT, -1.0, np.float32)
        blkid = np.zeros(EPT // 128, np.int32)
        eperm = np.full(EPT, -1, np.int64)
        for b in (0, 1):
            boff = b * EP[0]
            pos = 0
            for run, bk, npad in cores[core][b]:
                n = len(run)
                if n:
                    sl = slice(boff + pos, boff + pos + n)
                    srcrel[sl] = pad_row[run] - b * c.HALF
                    dstrel[sl] = dst_loc[run]
                    dstblk[sl] = (dst_loc[run] - bk * 128).astype(np.float32)
                    eperm[sl] = run
                blkid[(boff + pos) // 128: (boff + pos + n + npad) // 128] = bk
                pos += n + npad

        eft = np.zeros((c.FE, EPT), np.float32)
        real = eperm >= 0
        eft[:, real] = ef[eperm[real]].T

        nfT = np.zeros((c.FV, c.NBP), np.float32)
        nfT[:, : c.NB] = nf[core * c.NB: (core + 1) * c.NB].T
        gidc = np.full(c.NBP, -1.0, np.float32)
        gidc[: c.NB] = gid[core * c.NB: (core + 1) * c.NB].astype(np.float32)

        # degree-weight columns for the exact sum(z) decomposition
        wsrc = np.zeros(c.NBP, np.float32)
        wsrc[: c.NB] = outdeg[core * c.NB: (core + 1) * c.NB]
        wdst = np.zeros(c.NBP, np.float32)
        wdst[: c.NB] = indeg[core * c.NB: (core + 1) * c.NB]

        # per-core edge-feature projection sums: (sum_e ef[e]) @ Wef2[l]
        efsum = ef[eperm[real]].sum(axis=0)  # [41]
        efW = np.stack([efsum @ Wef2[l] for l in range(c.L)], axis=0)  # [L,128]

        # merged gather indices: per 2048-edge group, 2048 src rows
        # (offset by NBP into the combined [adst; agout_half] table) then
        # 2048 dst rows (adst lives at rows [0, NBP)).
        srcrel2 = c.NBP + srcrel
        gidx = np.empty(2 * EPT, np.int64)
        ng = EPT // (128 * c.GS)
        gsz = 128 * c.GS
        for gi in range(ng):
            base = gi * gsz
            gidx[2 * base: 2 * base + gsz] = srcrel2[base: base + gsz]
            gidx[2 * base + gsz: 2 * base + 2 * gsz] = dstrel[base: base + gsz]
        m = {
            "gidx": _wrap_idx16(gidx.astype(np.int16)),
            "dstblk": dstblk.reshape(-1, 128).T.copy(),
            "blkid": blkid.reshape(1, -1),
            "eft": eft.astype(np.bfloat16) if hasattr(np, "bfloat16") else eft,
            "nfT": nfT,
            "gidc": gidc.reshape(-1, 128).T.copy(),
            "wsrc": wsrc.reshape(-1, 128).T.astype(np.float16).copy(),
            "wdst": wdst.reshape(-1, 128).T.astype(np.float16).copy(),
            "efW": efW.reshape(1, -1).astype(np.float32),
        }
        in_maps.append(m)

    def to_bf16(x):
        import ml_dtypes
        return x.astype(ml_dtypes.bfloat16)

    shared = {
        "W_emb": np.asarray(inputs["W_emb"], np.float32),
        "g_emb": np.asarray(inputs["g_emb"], np.float32).reshape(1, E),
        "be_emb": np.asarray(inputs["be_emb"], np.float32).reshape(1, E),
        "Wsrc2": to_bf16(np.concatenate([Wm[:, :E, :], Ws[:, :E, :]], axis=2)),
        "Wdst2": to_bf16(np.concatenate([Wm[:, E:2 * E, :], Ws[:, E:2 * E, :]], axis=2)),
        "Wef2": to_bf16(Wef2),
        "gm": np.asarray(inputs["gm"], np.float32),
        "bem": np.asarray(inputs["bem"], np.float32),
        "gs": np.asarray(inputs["gs"], np.float32),
        "bes": np.asarray(inputs["bes"], np.float32),
        "gn": np.asarray(inputs["gn"], np.float32),
        "ben": np.asarray(inputs["ben"], np.float32),
        "Wf0": np.asarray(inputs["Wf0"], np.float32),
        "gf0": np.asarray(inputs["gf0"], np.float32).reshape(-1, 1),
        "bef0": np.asarray(inputs["bef0"], np.float32).reshape(-1, 1),
        "Wf1": np.asarray(inputs["Wf1"], np.float32),
        "gf1": np.asarray(inputs["gf1"], np.float32).reshape(-1, 1),
        "bef1": np.asarray(inputs["bef1"], np.float32).reshape(-1, 1),
        "Wt": np.asarray(inputs["Wt"], np.float32),
        "bt": np.asarray(inputs["bt"], np.float32).reshape(1, 1),
    }
    for m in in_maps:
        # eft conversion (numpy lacks bfloat16; use ml_dtypes)
        m["eft"] = to_bf16(np.asarray(m["eft"], np.float32))
        m.update(shared)
    return in_maps, EP


# ------------------------------------------------------------- kernel build
def build(cfg, EP, dbg=False):
    c = cfg
    EPT = EP[0] + EP[1]
    NCH = EPT // 128
    DVE = mybir.EngineType.DVE
    AF = mybir.ActivationFunctionType
    OP = mybir.AluOpType

    nc = bacc.Bacc("TRN2", target_bir_lowering=False, debug=False,
                   enable_asserts=False, num_devices=c.NC, num_swdge_queues=4)

    def din(name, shape, dt=FP):
        return nc.dram_tensor(name, shape, dt, kind="ExternalInput")

    t_gidx = din("gidx", [128, 2 * EPT // 16], mybir.dt.int16)
    t_dstblk = din("dstblk", [128, NCH], FP)
    t_blkid = din("blkid", [1, NCH], mybir.dt.int32)
    t_eft = din("eft", [c.FE, EPT], BF16)
    t_nfT = din("nfT", [c.FV, c.NBP])
    t_gidc = din("gidc", [128, c.NBLK])
    t_wsrc = din("wsrc", [128, c.NBLK], F16)
    t_wdst = din("wdst", [128, c.NBLK], F16)
    t_efW = din("efW", [1, c.L * c.ZF])
    t_Wemb = din("W_emb", [c.FV, c.E])
    t_gemb = din("g_emb", [1, c.E])
    t_beemb = din("be_emb", [1, c.E])
    t_Wsrc2 = din("Wsrc2", [c.L, c.E, c.ZF], BF16)
    t_Wdst2 = din("Wdst2", [c.L, c.E, c.ZF], BF16)
    t_Wef2 = din("Wef2", [c.L, c.FE, c.ZF], BF16)
    t_gm = din("gm", [c.L, c.E])
    t_bem = din("bem", [c.L, c.E])
    t_gs = din("gs", [c.L, c.E])
    t_bes = din("bes", [c.L, c.E])
    t_gn = din("gn", [c.L, c.E])
    t_ben = din("ben", [c.L, c.E])
    t_Wf0 = din("Wf0", [c.E, c.FC0])
    t_gf0 = din("gf0", [c.FC0, 1])
    t_bef0 = din("bef0", [c.FC0, 1])
    t_Wf1 = din("Wf1", [c.FC0, c.FC1])
    t_gf1 = din("gf1", [c.FC1, 1])
    t_bef1 = din("bef1", [c.FC1, 1])
    t_Wt = din("Wt", [c.E, 1])
    t_bt = din("bt", [1, 1])
    t_out = nc.dram_tensor("out", [1, c.NG], FP, kind="ExternalOutput")
    if dbg:
        t_dv = nc.dram_tensor("dbg_v", [128, c.NBP // 128 * c.E], FP, kind="ExternalOutput")
        t_dst0 = nc.dram_tensor("dbg_est0", [1, 2 * c.ZF], FP, kind="ExternalOutput")
        t_dagg = nc.dram_tensor("dbg_agg", [128, c.NBP // 128 * c.E], FP, kind="ExternalOutput")
        t_dvl = [nc.dram_tensor(f"dbg_vl{i}", [128, c.NBP // 128 * c.E], FP, kind="ExternalOutput")
                 for i in range(3)]
        t_dnst = [nc.dram_tensor(f"dbg_nst{i}", [1, 2 * c.E], FP, kind="ExternalOutput")
                  for i in range(3)]
        t_dz = nc.dram_tensor("dbg_z", [128, 4 * c.ZF], FP, kind="ExternalOutput")

    RG = [list(range(c.NC))]

    with tile.TileContext(nc) as tc, ExitStack() as es:
        dram = es.enter_context(tc.tile_pool(name="dram", bufs=1, space="DRAM"))
        zbuf = dram.tile([128, NCH, c.ZF], F16)
        comb = dram.tile([2, c.NBP + c.HALF, c.ZF], F16)
        est_in = [dram.tile([1, 2 * c.ZF], FP, name=f"est_in{i}") for i in range(c.L)]
        est_out = [dram.tile([1, 2 * c.ZF], FP, addr_space="Shared", name=f"est_out{i}")
                   for i in range(c.L)]
        nst_in = [dram.tile([1, 2 * c.E], FP, name=f"nst_in{i}") for i in range(c.L + 1)]
        nst_out = [dram.tile([1, 2 * c.E], FP, addr_space="Shared", name=f"nst_out{i}")
                   for i in range(c.L + 1)]
        agin_l = [dram.tile([c.NBP, c.ZF], F16, name=f"agin{i}") for i in range(c.L)]
        agout_l = [dram.tile([c.NT, c.ZF], F16, addr_space="Shared", name=f"agout{i}")
                   for i in range(c.L)]
        ro_in = dram.tile([c.E + 1, c.NG], FP)
        ro_out = dram.tile([c.E + 1, c.NG], FP, addr_space="Shared")

        konst = es.enter_context(tc.tile_pool(name="konst", bufs=1))
        iotaF = konst.tile([128, 256], FP)
        iota16 = konst.tile([128, 128], F16)
        identF = konst.tile([128, 128], FP)
        ones_row = konst.tile([1, 128], FP)
        ones_col = konst.tile([128, 1], FP)
        epsT = konst.tile([1, 1], FP)
        epsC = konst.tile([128, 1], FP)
        padmask = konst.tile([128, 1], FP)
        with tc.tile_pool(name="ksetup", bufs=1) as ks:
            ii = ks.tile([128, 256], mybir.dt.int32)
            nc.gpsimd.iota(ii[:], pattern=[[1, 256]], base=0, channel_multiplier=0)
            nc.vector.tensor_copy(iotaF[:], ii[:])
            nc.vector.tensor_copy(iota16[:], ii[:, :128])
            ip = ks.tile([128, 1], mybir.dt.int32)
            nc.gpsimd.iota(ip[:], pattern=[[1, 1]], base=0, channel_multiplier=1)
            ipf = ks.tile([128, 1], FP)
            nc.vector.tensor_copy(ipf[:], ip[:])
            nc.vector.tensor_scalar(identF[:], iotaF[:, :128], ipf[:], None, OP.is_equal)
            nc.vector.tensor_scalar(padmask[:], ipf[:], float(c.NB % 128), None, OP.is_lt)
        nc.vector.memset(ones_row[:], 1.0)
        nc.vector.memset(ones_col[:], 1.0)
        nc.vector.memset(epsT[:], c.EPS)
        nc.vector.memset(epsC[:], c.EPS)

        state = es.enter_context(tc.tile_pool(name="state", bufs=1))
        v_sb = state.tile([128, c.NBLK, c.E], FP)
        agg_sb = state.tile([128, c.NBLK, c.E], FP)
        blkid_sb = state.tile([1, NCH], mybir.dt.int32)
        dstblk_sb = state.tile([128, NCH], FP)
        gid_sb = state.tile([128, c.NBLK], FP)
        wsrc_sb = state.tile([128, c.NBLK], F16)
        wdst_sb = state.tile([128, c.NBLK], F16)
        efW_sb = state.tile([1, c.L * c.ZF], FP)
        sz_sb = state.tile([1, c.ZF], FP)
        s_g = state.tile([128, c.GS, c.ZF], F16)
        t_g = state.tile([128, c.GS, c.ZF], F16)
        nc.sync.dma_start(blkid_sb[:], t_blkid[:])
        nc.sync.dma_start(dstblk_sb[:], t_dstblk[:])
        nc.sync.dma_start(gid_sb[:], t_gidc[:])
        nc.sync.dma_start(wsrc_sb[:], t_wsrc[:])
        nc.sync.dma_start(wdst_sb[:], t_wdst[:])
        nc.sync.dma_start(efW_sb[:], t_efW[:])

        wts = es.enter_context(tc.tile_pool(name="wts", bufs=1))
        Wsrc2_sb = wts.tile([c.E, c.L * c.ZF], BF16)
        Wdst2_sb = wts.tile([c.E, c.L * c.ZF], BF16)
        Wef2_sb = wts.tile([c.FE, c.L * c.ZF], BF16)
        for l in range(c.L):
            nc.sync.dma_start(Wsrc2_sb[:, l * c.ZF:(l + 1) * c.ZF], t_Wsrc2[l])
            nc.sync.dma_start(Wdst2_sb[:, l * c.ZF:(l + 1) * c.ZF], t_Wdst2[l])
            nc.sync.dma_start(Wef2_sb[:, l * c.ZF:(l + 1) * c.ZF], t_Wef2[l])

        def bn_fold(pool, sums, F, count, g_ap, be_ap):
            st = pool.tile([1, 2 * F], FP, name=f"bnf{nc.next_id()}")
            mean = pool.tile([1, F], FP, name=f"bnm{nc.next_id()}")
            var = pool.tile([1, F], FP, name=f"bnv{nc.next_id()}")
            nc.scalar.mul(mean[:], sums[:, 0:F], 1.0 / count)
            nc.scalar.mul(var[:], sums[:, F:2 * F], 1.0 / count)
            m2 = pool.tile([1, F], FP, name=f"bn2{nc.next_id()}")
            nc.vector.tensor_mul(m2[:], mean[:], mean[:])
            nc.vector.tensor_sub(var[:], var[:], m2[:])
            nc.scalar.activation(var[:], var[:], AF.Ln, bias=epsT[0:1, 0:1])
            nc.scalar.activation(var[:], var[:], AF.Exp, scale=-0.5)
            nc.vector.tensor_mul(st[:, 0:F], g_ap, var[:])
            nc.vector.tensor_mul(mean[:], mean[:], st[:, 0:F])
            nc.vector.tensor_sub(st[:, F:2 * F], be_ap, mean[:])
            return st

        def bcast_row(pool, psum_pool, row_ap, W, name):
            ps = psum_pool.tile([128, W], FP, name=f"ps{name}")
            nc.tensor.matmul(ps[:], ones_row[:, :], row_ap, start=True, stop=True)
            sb = pool.tile([128, W], FP, name=name)
            nc.scalar.copy(sb[:], ps[:])
            return sb

        def zero_vpad():
            cb = c.NB // 128
            nc.vector.tensor_scalar(v_sb[:, cb, :], v_sb[:, cb, :],
                                    padmask[:], None, OP.mult)

        # ---------------------------------------------------- embedding
        with tc.tile_pool(name="emb", bufs=1) as emb, \
             tc.tile_pool(name="embw", bufs=2) as embw, \
             tc.tile_pool(name="embp", bufs=2, space="PSUM") as embp, \
             tc.tile_pool(name="embs", bufs=1, space="PSUM") as embs:
            nfT_sb = emb.tile([c.FV, c.NBP], FP)
            nc.sync.dma_start(nfT_sb[:], t_nfT[:])
            Wemb_sb = emb.tile([c.FV, c.E], FP)
            nc.sync.dma_start(Wemb_sb[:], t_Wemb[:])
            z0 = emb.tile([128, c.NBLK, c.E], FP)
            ssum = embs.tile([1, c.E], FP)
            ssq = embs.tile([1, c.E], FP)
            for ch in range(c.NBLK):
                ps = embp.tile([128, c.E], FP, name="embz")
                nc.tensor.matmul(ps[:], nfT_sb[:, ch * 128:(ch + 1) * 128],
                                 Wemb_sb[:], start=True, stop=True)
                nc.scalar.copy(z0[:, ch, :], ps[:])
                sq = embw.tile([128, c.E], FP, name="embsq")
                nc.vector.tensor_mul(sq[:], z0[:, ch, :], z0[:, ch, :])
                nc.tensor.matmul(ssum[:], ones_col[:, :], z0[:, ch, :],
                                 start=(ch == 0), stop=(ch == c.NBLK - 1))
                nc.tensor.matmul(ssq[:], ones_col[:, :], sq[:],
                                 start=(ch == 0), stop=(ch == c.NBLK - 1))
            stat = emb.tile([1, 2 * c.E], FP)
            nc.vector.tensor_copy(stat[:, 0:c.E], ssum[:])
            nc.vector.tensor_copy(stat[:, c.E:], ssq[:])
            nc.sync.dma_start(nst_in[c.L][:], stat[:])
            nc.gpsimd.collective_compute(
                "AllReduce", OP.add, replica_groups=RG,
                ins=[nst_in[c.L].opt()], outs=[nst_out[c.L].opt()])
            rstat = emb.tile([1, 2 * c.E], FP)
            nc.sync.dma_start(rstat[:], nst_out[c.L][:])
            gemb_sb = emb.tile([1, c.E], FP)
            beemb_sb = emb.tile([1, c.E], FP)
            nc.sync.dma_start(gemb_sb[:], t_gemb[:])
            nc.sync.dma_start(beemb_sb[:], t_beemb[:])
            st = bn_fold(emb, rstat, c.E, c.N, gemb_sb[:], beemb_sb[:])
            stb = bcast_row(emb, embp, st[:], 2 * c.E, "embst")
            for ch in range(c.NBLK):
                u = embw.tile([128, c.E], FP, name="embu")
                nc.vector.tensor_mul(u[:], z0[:, ch, :], stb[:, 0:c.E])
                nc.vector.tensor_add(u[:], u[:], stb[:, c.E:])
                # silu(u) = u * exp(-ln(1 + exp(-u)))
                e = embw.tile([128, c.E], FP, name="embe")
                nc.scalar.activation(e[:], u[:], AF.Exp, scale=-1.0)
                nc.scalar.activation(e[:], e[:], AF.Ln, bias=1.0)
                nc.scalar.activation(e[:], e[:], AF.Exp, scale=-1.0)
                nc.vector.tensor_mul(v_sb[:, ch, :], u[:], e[:])
            zero_vpad()
        if dbg:
            nc.sync.dma_start(t_dv[:], v_sb[:].rearrange("p b f -> p (b f)"))

        # ---------------------------------------------------- conv layers
        gq = 0
        for l in range(c.L):
            # ---- phase A: projection tables (f16) + exact sum(z) dots
            with tc.tile_pool(name="phA", bufs=2) as pa, \
                 tc.tile_pool(name="phAp", bufs=2, space="PSUM") as pap, \
                 tc.tile_pool(name="phAo", bufs=2, space="PSUM") as pao, \
                 tc.tile_pool(name="phAs", bufs=1, space="PSUM") as pas:
                asrc_sb = pa.tile([128, c.NBLK, c.ZF], F16, bufs=1)
                adst_sb = pa.tile([128, c.NBLK, c.ZF], F16, bufs=1)
                for ch in range(c.NBLK):
                    vt_ps = pap.tile([c.E, 128], FP, name="vtps")
                    nc.tensor.transpose(vt_ps[:], v_sb[:, ch, :], identF[:])
                    vt = pa.tile([c.E, 128], BF16, name="vt")
                    nc.scalar.copy(vt[:], vt_ps[:])
                    a1 = pao.tile([128, c.ZF], FP, name="a1")
                    nc.tensor.matmul(a1[:], vt[:], Wsrc2_sb[:, l * c.ZF:(l + 1) * c.ZF],
                                     start=True, stop=True)
                    nc.scalar.copy(asrc_sb[:, ch, :], a1[:])
                    a2 = pao.tile([128, c.ZF], FP, name="a2")
                    nc.tensor.matmul(a2[:], vt[:], Wdst2_sb[:, l * c.ZF:(l + 1) * c.ZF],
                                     start=True, stop=True)
                    nc.vector.tensor_copy(adst_sb[:, ch, :], a2[:])
                nc.sync.dma_start(
                    agin_l[l][:].rearrange("(b p) f -> p b f", p=128), asrc_sb[:])
                for b in (0, 1):
                    nc.sync.dma_start(
                        comb[b, 0:c.NBP, :].rearrange("(blk p) f -> p blk f", p=128),
                        adst_sb[:])
                # exact sum(z): degree-weighted dots over local shards
                szsrc = pas.tile([1, c.ZF], FP)
                szdst = pas.tile([1, c.ZF], FP)
                for ch in range(c.NBLK):
                    nc.tensor.matmul(szsrc[:], wsrc_sb[:, ch:ch + 1],
                                     asrc_sb[:, ch, :],
                                     start=(ch == 0), stop=(ch == c.NBLK - 1))
                    nc.tensor.matmul(szdst[:], wdst_sb[:, ch:ch + 1],
                                     adst_sb[:, ch, :],
                                     start=(ch == 0), stop=(ch == c.NBLK - 1))
                nc.vector.tensor_copy(sz_sb[:], szsrc[:])
                nc.vector.tensor_add(sz_sb[:], sz_sb[:], szdst[:])
                nc.vector.tensor_add(sz_sb[:], sz_sb[:],
                                     efW_sb[:, l * c.ZF:(l + 1) * c.ZF])
            nc.gpsimd.collective_compute(
                "AllGather", OP.bypass, replica_groups=RG,
                ins=[agin_l[l].opt()], outs=[agout_l[l].opt()])
            # stage the two halves behind their adst copies in the combined table
            for b in (0, 1):
                nc.sync.dma_start(comb[b, c.NBP:, :],
                                  agout_l[l][b * c.HALF:(b + 1) * c.HALF, :])

            # ---- pass 1: z assembly + f16 spill + Gram stats
            with tc.tile_pool(name="p1idx", bufs=4) as pidx, \
                 tc.tile_pool(name="p1g", bufs=4) as pg, \
                 tc.tile_pool(name="p1z", bufs=3) as pz, \
                 tc.tile_pool(name="p1st", bufs=1) as p1st, \
                 tc.tile_pool(name="p1zp", bufs=1, space="PSUM") as pzp, \
                 tc.tile_pool(name="p1gr", bufs=1, space="PSUM") as pgr:
                gram = pgr.tile([128, c.ZF], FP)
                first_mm = [True]
                for b in (0, 1):
                    nchb = EP[b] // 128
                    base_ch = (0 if b == 0 else EP[0] // 128)
                    for g0 in range(0, nchb, c.GS):
                        gs = c.GS
                        ni = gs * 128
                        coff = base_ch + g0
                        last_grp = (b == 1 and g0 + c.GS >= nchb)
                        idxs_t = pidx.tile([128, c.GS * 16], mybir.dt.int16, name="idxs")
                        nc.sync.dma_start(idxs_t[:],
                                          t_gidx[:, coff * 16:coff * 16 + gs * 16])
                        gath = pg.tile([128, 2 * c.GS, c.ZF], F16, name="gath")
                        nc.gpsimd.dma_gather(
                            gath[:], comb[b],
                            idxs_t[:], num_idxs=2 * ni, num_idxs_reg=2 * ni,
                            elem_size=c.ZF, queue_num=gq % 4, single_packet=False)
                        gq += 1
                        ef_t = pg.tile([c.FE, c.GS * 128], BF16, name="eft")
                        nc.sync.dma_start(ef_t[:, :ni],
                                          t_eft[:, coff * 128:coff * 128 + ni])
                        zp = pzp.tile([128, c.GS, c.ZF], FP, name="zp")
                        for j in range(gs):
                            nc.tensor.matmul(zp[:, j, :], ef_t[:, j * 128:(j + 1) * 128],
                                             Wef2_sb[:, l * c.ZF:(l + 1) * c.ZF],
                                             start=True, stop=True)
                        pef_t = pz.tile([128, c.GS, c.ZF], F16, name="peft")
                        nc.scalar.copy(pef_t[:, :gs, :], zp[:, :gs, :])
                        t1 = pz.tile([128, c.GS, c.ZF], F16, name="t1")
                        nc.vector.tensor_add(t1[:, :gs, :], gath[:, :gs, :],
                                             gath[:, gs:2 * gs, :])
                        z_t = pz.tile([128, c.GS, c.ZF], F16, name="zt")
                        nc.vector.tensor_add(z_t[:, :gs, :], t1[:, :gs, :],
                                             pef_t[:, :gs, :])
                        nc.sync.dma_start(zbuf[:, coff:coff + gs, :], z_t[:, :gs, :])
                        for j in range(gs):
                            nc.tensor.matmul(gram[:], z_t[:, j, :], z_t[:, j, :],
                                             start=first_mm[0],
                                             stop=(last_grp and j == gs - 1))
                            first_mm[0] = False
                # stats: diag(gram) -> row; assemble [sum(z) | sum(z^2)]
                gI = p1st.tile([128, c.ZF], FP)
                nc.vector.tensor_mul(gI[:], gram[:], identF[:])
                with tc.tile_pool(name="p1sq", bufs=1, space="PSUM") as psq:
                    szq = psq.tile([1, c.ZF], FP)
                    nc.tensor.matmul(szq[:], ones_col[:, :], gI[:], start=True, stop=True)
                    stat = p1st.tile([1, 2 * c.ZF], FP)
                    nc.vector.tensor_copy(stat[:, :c.ZF], sz_sb[:])
                    nc.vector.tensor_copy(stat[:, c.ZF:], szq[:])
                    nc.sync.dma_start(est_in[l][:], stat[:])

            nc.gpsimd.collective_compute(
                "AllReduce", OP.add, replica_groups=RG,
                ins=[est_in[l].opt()], outs=[est_out[l].opt()])

            # ---- fold BN affine, broadcast s/t to [128, GS, ZF] f16
            with tc.tile_pool(name="pmid", bufs=1) as pm, \
                 tc.tile_pool(name="pmidp", bufs=1, space="PSUM") as pmp:
                rstat = pm.tile([1, 2 * c.ZF], FP)
                nc.sync.dma_start(rstat[:], est_out[l][:])
                gms = pm.tile([1, 2 * c.E], FP)
                nc.sync.dma_start(gms[:, :c.E], t_gm[l:l + 1, :])
                nc.sync.dma_start(gms[:, c.E:], t_gs[l:l + 1, :])
                bms = pm.tile([1, 2 * c.E], FP)
                nc.sync.dma_start(bms[:, :c.E], t_bem[l:l + 1, :])
                nc.sync.dma_start(bms[:, c.E:], t_bes[l:l + 1, :])
                if dbg and l == 0:
                    nc.sync.dma_start(t_dst0[:], rstat[:])
                    zdbg = pm.tile([128, 4, c.ZF], F16, name="zdbg")
                    nc.sync.dma_start(zdbg[:], zbuf[:, 0:4, :])
                    zdbgf = pm.tile([128, 4 * c.ZF], FP, name="zdbgf")
                    nc.vector.tensor_copy(zdbgf[:], zdbg[:].rearrange("p b f -> p (b f)"))
                    nc.sync.dma_start(t_dz[:], zdbgf[:])
                st = bn_fold(pm, rstat, c.ZF, c.M, gms[:], bms[:])
                stb = bcast_row(pm, pmp, st[:], 2 * c.ZF, "edgest")
                # negate the sigmoid (m) half so exp(u) = exp(-um) there
                nc.vector.tensor_scalar(stb[:, 0:c.E], stb[:, 0:c.E],
                                        -1.0, None, OP.mult)
                nc.vector.tensor_scalar(stb[:, c.ZF:c.ZF + c.E],
                                        stb[:, c.ZF:c.ZF + c.E],
                                        -1.0, None, OP.mult)
                for j in range(c.GS):
                    nc.vector.tensor_copy(s_g[:, j, :], stb[:, 0:c.ZF])
                    nc.vector.tensor_copy(t_g[:, j, :], stb[:, c.ZF:])

            # ---- pass 2: activations + PSUM-resident scatter
            with tc.tile_pool(name="p2z", bufs=3) as p2z, \
                 tc.tile_pool(name="p2ap", bufs=1, space="PSUM") as p2ap:
                agg_ps = p2ap.tile([128, c.NBLK, c.E], FP)
                agg_flat = agg_ps[:].rearrange("p b f -> p (b f)")
                nc.vector.memset(agg_ps[:], 0.0)
                n_sc = [0]
                total_sc = NCH
                for b in (0, 1):
                    nchb = EP[b] // 128
                    base_ch = (0 if b == 0 else EP[0] // 128)
                    for g0 in range(0, nchb, c.GS):
                        gs = min(c.GS, nchb - g0)
                        coff = base_ch + g0
                        z_t = p2z.tile([128, c.GS, c.ZF], F16, name="z2t")
                        nc.sync.dma_start(z_t[:, :gs, :], zbuf[:, coff:coff + gs, :])
                        u = p2z.tile([128, c.GS, c.ZF], F16, name="u")
                        nc.vector.tensor_mul(u[:, :gs, :], z_t[:, :gs, :], s_g[:, :gs, :])
                        nc.vector.tensor_add(u[:, :gs, :], u[:, :gs, :], t_g[:, :gs, :])
                        uc = p2z.tile([128, c.GS, c.ZF], F16, name="uc")
                        nc.vector.tensor_scalar_min(uc[:, :gs, :], u[:, :gs, :], 11.0)
                        e_t = p2z.tile([128, c.GS, c.ZF], F16, name="et")
                        nc.scalar.activation(e_t[:, :gs, :], uc[:, :gs, :], AF.Exp)
                        lnm = p2z.tile([128, c.GS, c.E], F16, name="lnm")
                        nc.scalar.activation(lnm[:, :gs, :], e_t[:, :gs, 0:c.E],
                                             AF.Ln, bias=1.0)
                        sg_t = p2z.tile([128, c.GS, c.E], F16, name="sgt")
                        nc.scalar.activation(sg_t[:, :gs, :], lnm[:, :gs, :],
                                             AF.Exp, scale=-1.0)
                        sp_t = p2z.tile([128, c.GS, c.E], F16, name="spt")
                        nc.scalar.activation(sp_t[:, :gs, :], e_t[:, :gs, c.E:],
                                             AF.Ln, bias=1.0)
                        # softplus(u) == u at f16 precision for u > 11 (clamped above)
                        nc.vector.tensor_tensor(sp_t[:, :gs, :], sp_t[:, :gs, :],
                                                u[:, :gs, c.E:], OP.max)
                        h = p2z.tile([128, c.GS, c.E], F16, name="h")
                        nc.vector.tensor_mul(h[:, :gs, :], sg_t[:, :gs, :],
                                             sp_t[:, :gs, :])
                        ind_t = p2z.tile([128, c.GS, 128], F16, name="ind")
                        for j in range(gs):
                            ch = coff + j
                            nc.vector.tensor_scalar(ind_t[:, j, :], iota16[:, :],
                                                    dstblk_sb[:, ch:ch + 1], None,
                                                    OP.is_equal)
                            r = nc.alloc_registers(engines=[mybir.EngineType.PE])
                            nc.regs_load(r, blkid_sb[0:1, ch:ch + 1])
                            bv = nc.snap(r, donate=True, min_val=0, max_val=c.NBLK - 1)
                            n_sc[0] += 1
                            nc.tensor.matmul(
                                agg_flat[:, bass.ts(bv, c.E)],
                                ind_t[:, j, :], h[:, j, :],
                                start=False, stop=(n_sc[0] == total_sc),
                                skip_group_check=True)
                # drain agg PSUM -> SBUF
                for o in range(0, c.NBLK * c.E, 512):
                    w = min(512, c.NBLK * c.E - o)
                    nc.scalar.copy(
                        agg_sb[:].rearrange("p b f -> p (b f)")[:, o:o + w],
                        agg_flat[:, o:o + w])

            if dbg and l == 0:
                nc.sync.dma_start(t_dagg[:], agg_sb[:].rearrange("p b f -> p (b f)"))

            # ---- node BN + update
            with tc.tile_pool(name="nod", bufs=1) as nod, \
                 tc.tile_pool(name="nodw", bufs=2) as nodw, \
                 tc.tile_pool(name="nodp", bufs=2, space="PSUM") as nodp, \
                 tc.tile_pool(name="nods", bufs=1, space="PSUM") as nods:
                nsum = nods.tile([1, c.E], FP)
                nssq = nods.tile([1, c.E], FP)
                for ch in range(c.NBLK):
                    sq = nodw.tile([128, c.E], FP, name="nsq")
                    nc.vector.tensor_mul(sq[:], agg_sb[:, ch, :], agg_sb[:, ch, :])
                    nc.tensor.matmul(nsum[:], ones_col[:, :], agg_sb[:, ch, :],
                                     start=(ch == 0), stop=(ch == c.NBLK - 1))
                    nc.tensor.matmul(nssq[:], ones_col[:, :], sq[:],
                                     start=(ch == 0), stop=(ch == c.NBLK - 1))
                stat = nod.tile([1, 2 * c.E], FP)
                nc.vector.tensor_copy(stat[:, :c.E], nsum[:])
                nc.vector.tensor_copy(stat[:, c.E:], nssq[:])
                nc.sync.dma_start(nst_in[l][:], stat[:])
                nc.gpsimd.collective_compute(
                    "AllReduce", OP.add, replica_groups=RG,
                    ins=[nst_in[l].opt()], outs=[nst_out[l].opt()])
                rstat = nod.tile([1, 2 * c.E], FP)
                nc.sync.dma_start(rstat[:], nst_out[l][:])
                gn_sb = nod.tile([1, c.E], FP)
                ben_sb = nod.tile([1, c.E], FP)
                nc.sync.dma_start(gn_sb[:], t_gn[l:l + 1, :])
                nc.sync.dma_start(ben_sb[:], t_ben[l:l + 1, :])
                st = bn_fold(nod, rstat, c.E, c.N, gn_sb[:], ben_sb[:])
                stb = bcast_row(nod, nodp, st[:], 2 * c.E, "nodst")
                for ch in range(c.NBLK):
                    u = nodw.tile([128, c.E], FP, name="nu")
                    nc.vector.tensor_mul(u[:], agg_sb[:, ch, :], stb[:, 0:c.E])
                    nc.vector.tensor_add(u[:], u[:], stb[:, c.E:])
                    nc.vector.tensor_add(u[:], u[:], v_sb[:, ch, :])
                    # softplus(u) = ln(1 + exp(u)); u is comfortably < 80
                    e = nodw.tile([128, c.E], FP, name="ne")
                    nc.scalar.activation(e[:], u[:], AF.Exp)
                    nc.scalar.activation(v_sb[:, ch, :], e[:], AF.Ln, bias=1.0)
                zero_vpad()
                if dbg:
                    nc.sync.dma_start(t_dvl[l][:], v_sb[:].rearrange("p b f -> p (b f)"))
                    nc.sync.dma_start(t_dnst[l][:], rstat[:])

        # ---------------------------------------------------- readout
        with tc.tile_pool(name="ro", bufs=1) as ro, \
             tc.tile_pool(name="row", bufs=2) as row, \
             tc.tile_pool(name="rop", bufs=1, space="PSUM") as rop, \
             tc.tile_pool(name="ros", bufs=1, space="PSUM") as ros:
            psums = ros.tile([c.E, c.NG], FP)
            pcnt = ros.tile([1, c.NG], FP)
            for ch in range(c.NBLK):
                gind = row.tile([128, c.NG], FP, name="gind")
                nc.vector.tensor_scalar(gind[:], iotaF[:, :c.NG],
                                        gid_sb[:, ch:ch + 1], None, OP.is_equal)
                nc.tensor.matmul(psums[:], v_sb[:, ch, :], gind[:],
                                 start=(ch == 0), stop=(ch == c.NBLK - 1))
                nc.tensor.matmul(pcnt[:], ones_col[:, :], gind[:],
                                 start=(ch == 0), stop=(ch == c.NBLK - 1))
            acc = ro.tile([c.E + 1, c.NG], FP)
            nc.scalar.copy(acc[0:c.E, :], psums[:])
            nc.scalar.copy(acc[c.E:c.E + 1, :], pcnt[:])
            nc.sync.dma_start(ro_in[:], acc[:])
            nc.gpsimd.collective_compute(
                "AllReduce", OP.add, replica_groups=RG,
                ins=[ro_in.opt()], outs=[ro_out.opt()])
            racc = ro.tile([c.E + 1, c.NG], FP)
            nc.sync.dma_start(racc[:], ro_out[:])
            cnt = ro.tile([1, c.NG], FP)
            nc.vector.tensor_scalar_max(cnt[:], racc[c.E:c.E + 1, :], 1.0)
            nc.scalar.activation(cnt[:], cnt[:], AF.Ln)
            nc.scalar.activation(cnt[:], cnt[:], AF.Exp, scale=-1.0)
            rcb_ps = rop.tile([c.E, c.NG], FP, name="rcb", tag="rosc")
            nc.tensor.matmul(rcb_ps[:], ones_row[:, 0:c.E], cnt[:], start=True, stop=True)
            vs = ro.tile([c.E, c.NG], FP)
            nc.vector.tensor_tensor(vs[:], racc[0:c.E, :], rcb_ps[:], OP.mult)

            def fc_bn_silu(pool, psum_pool, x_sb, W_ap, K, Fo, g_t, be_t, nm):
                ps = psum_pool.tile([Fo, c.NG], FP, name=f"fc{nm}", tag="rosc")
                W_sb = pool.tile([K, Fo], FP, name=f"W{nm}")
                nc.sync.dma_start(W_sb[:], W_ap)
                nc.tensor.matmul(ps[:], W_sb[:], x_sb[:], start=True, stop=True)
                g_sb = pool.tile([Fo, 1], FP, name=f"g{nm}")
                be_sb = pool.tile([Fo, 1], FP, name=f"be{nm}")
                nc.sync.dma_start(g_sb[:], g_t[:])
                nc.sync.dma_start(be_sb[:], be_t[:])
                x_sbc = pool.tile([Fo, c.NG], FP, name=f"x{nm}")
                nc.scalar.copy(x_sbc[:], ps[:])
                sums = pool.tile([Fo, 1], FP, name=f"su{nm}")
                nc.vector.tensor_reduce(sums[:], x_sbc[:], mybir.AxisListType.X, OP.add)
                sq = pool.tile([Fo, c.NG], FP, name=f"sq{nm}")
                nc.vector.tensor_mul(sq[:], x_sbc[:], x_sbc[:])
                ssq = pool.tile([Fo, 1], FP, name=f"sl{nm}")
                nc.vector.tensor_reduce(ssq[:], sq[:], mybir.AxisListType.X, OP.add)
                mean = pool.tile([Fo, 1], FP, name=f"mn{nm}")
                nc.scalar.mul(mean[:], sums[:], 1.0 / c.NG)
                var = pool.tile([Fo, 1], FP, name=f"vr{nm}")
                nc.scalar.mul(var[:], ssq[:], 1.0 / c.NG)
                m2 = pool.tile([Fo, 1], FP, name=f"m2{nm}")
                nc.vector.tensor_mul(m2[:], mean[:], mean[:])
                nc.vector.tensor_sub(var[:], var[:], m2[:])
                nc.scalar.activation(var[:], var[:], AF.Ln, bias=epsC[0:Fo, :])
                nc.scalar.activation(var[:], var[:], AF.Exp, scale=-0.5)
                s_col = pool.tile([Fo, 1], FP, name=f"sc{nm}")
                nc.vector.tensor_mul(s_col[:], g_sb[:], var[:])
                t_col = pool.tile([Fo, 1], FP, name=f"tc{nm}")
                nc.vector.tensor_mul(t_col[:], mean[:], s_col[:])
                nc.vector.tensor_sub(t_col[:], be_sb[:], t_col[:])
                u = pool.tile([Fo, c.NG], FP, name=f"u{nm}")
                nc.scalar.activation(u[:], x_sbc[:], AF.Identity,
                                     bias=t_col[:], scale=s_col[:])
                # silu(u) = u * exp(-ln(1 + exp(-u)))
                e = pool.tile([Fo, c.NG], FP, name=f"e{nm}")
                nc.scalar.activation(e[:], u[:], AF.Exp, scale=-1.0)
                nc.scalar.activation(e[:], e[:], AF.Ln, bias=1.0)
                nc.scalar.activation(e[:], e[:], AF.Exp, scale=-1.0)
                out = pool.tile([Fo, c.NG], FP, name=f"o{nm}")
                nc.vector.tensor_mul(out[:], u[:], e[:])
                return out

            z1 = fc_bn_silu(ro, rop, vs, t_Wf0[:], c.E, c.FC0, t_gf0, t_bef0, "0")
            z2 = fc_bn_silu(ro, rop, z1, t_Wf1[:], c.FC0, c.FC1, t_gf1, t_bef1, "1")
            Wt_sb = ro.tile([c.E, 1], FP)
            nc.sync.dma_start(Wt_sb[:], t_Wt[:])
            hd = rop.tile([1, c.NG], FP, name="hd", tag="rosc")
            nc.tensor.matmul(hd[:], Wt_sb[:], z2[:], start=True, stop=True)
            bt_sb = ro.tile([1, 1], FP)
            nc.sync.dma_start(bt_sb[:], t_bt[:])
            res = ro.tile([1, c.NG], FP)
            nc.vector.tensor_scalar(res[:], hd[:], bt_sb[0:1, 0:1], None, OP.add)
            nc.sync.dma_start(t_out[:], res[:])

    nc.compile()
    return nc


# ------------------------------------------------------------------ driver
_CACHE = {}


def kernel(**inputs):
    cfg = Cfg(int(inputs["node_feats"].shape[0]),
              int(inputs["src"].shape[0]), 256)
    in_maps, EP = preprocess(inputs, cfg)
    key = (cfg.N, cfg.M, tuple(EP))
    if key not in _CACHE:
        _CACHE[key] = build(cfg, EP)
    nc = _CACHE[key]
    res = bass_utils.run_bass_kernel_spmd(
        nc, in_maps, core_ids=list(range(cfg.NC)), trace=False)
    out = np.asarray(res.results[0]["out"], np.float32)
    return out.reshape(cfg.NG, 1)


# revision 17
# speedup vs baseline: 1.7380x; 1.2425x over previous
"""CGCNN (gnn_message_passing) Trainium2 kernel — 8-core SPMD, v2.

Strategy (v2, redesigned from the 7.9ms baseline traced as DMA-bound):
  - Nodes partitioned contiguously across 8 cores (6250/core, padded to 6272);
    edges assigned to the core owning their dst node, sorted by dst, grouped
    into 128-edge chunks that never cross a 128-node dst block.
  - Per conv layer each core computes f16 projection tables
      A_src = v @ [Wm_src|Ws_src]  (AllGathered, f16: half the bytes)
      A_dst = v @ [Wm_dst|Ws_dst]  (local DRAM, f16)
    and gathers them per edge via SWDGE (256B descriptors).
  - z[e] = A_src[src] + A_dst[dst] + ef[e] @ Wef (bf16 matmul into PSUM,
    drained to SBUF by the Act engine; two f16 DVE adds).  z spilled f16.
  - BN stats without big accumulators: sum(z) is computed EXACTLY from
    host-precomputed global degree weights dotted with the local A_src/A_dst
    shards (PE matmuls); sum(z^2) comes from a layer-wide Gram accumulation
    z^T z in PSUM whose diagonal is extracted once per layer. Tiny AllReduce.
  - Pass 2 reloads z, applies the folded BN affine (two f16 DVE ops) and
    sigmoid*softplus built from one act table:
      e = exp(u); sigma = recip(1+e_m) (f16); sp = ln(e_s + 1) (bias fold).
    Scatter-sum via per-chunk indicator matmuls generated ON-CHIP
    (iota==dstblk compare), accumulating straight into a PSUM-resident
    agg[128, NBLK, 64] region through register-offset matmul outputs.
  - Node BN: local sums + tiny AllReduce.  Readout replicated per core.
"""

import sys
import os
from contextlib import ExitStack

sys.path.insert(0, "/opt/trn_rl_repo")

import numpy as np

import concourse.bass as bass
import concourse.bacc as bacc
import concourse.tile as tile
from concourse import mybir, bass_utils
import concourse.hw_specs as hw_specs

FP = mybir.dt.float32
F16 = mybir.dt.float16
BF16 = mybir.dt.bfloat16

# Restrict activation-table selection to one set so the scalar engine never
# reloads tables (everything is built from Exp/Ln/Relu/Identity/Copy).
_KEEP_TABLES = {"natural_log_exp_and_others"}


def _patched_tables(arch):
    t = hw_specs.get_activation_tables(arch)
    return {k: (v if k in _KEEP_TABLES else set()) for k, v in t.items()}


bacc.get_activation_tables = _patched_tables


# ---------------------------------------------------------------- config
class Cfg:
    def __init__(self, N, M, NG):
        self.NC = 8
        self.N, self.M, self.NG = N, M, NG
        self.FV, self.FE, self.E, self.L = 92, 41, 64, 3
        self.FC0, self.FC1 = 128, 64
        self.ZF = 128                       # z width = 2*E
        self.NB = N // self.NC              # real nodes per core
        self.NBP = -(-(self.NB + 1) // 128) * 128  # padded (>= NB+1: zero row)
        self.NBLK = self.NBP // 128
        self.NT = self.NBP * self.NC
        self.HALF = self.NT // 2
        assert self.HALF - 1 < 32768
        assert self.NBP > self.NB
        self.GS = 16                        # chunks per group (2048 edges)
        self.EPS = 1e-5
        self.CPB = None                     # chunks per (block,bucket) cell


# ---------------------------------------------------------- preprocessing
def _wrap_idx16(idx):
    a = idx.reshape(-1, 16).T.astype(np.int16)
    return np.tile(a, (8, 1))


def preprocess(inputs, cfg):
    c = cfg
    src = np.asarray(inputs["src"]).astype(np.int64)
    dst = np.asarray(inputs["dst"]).astype(np.int64)
    ef = np.asarray(inputs["edge_feats"], np.float32)
    nf = np.asarray(inputs["node_feats"], np.float32)
    gid = np.asarray(inputs["graph_ids"]).astype(np.int64)

    pad_row = (src // c.NB) * c.NBP + (src % c.NB)
    owner = dst // c.NB
    dst_loc = dst - owner * c.NB

    # global degree weights (counts over ALL edges)
    outdeg = np.bincount(src, minlength=c.N).astype(np.float32)
    indeg = np.bincount(dst, minlength=c.N).astype(np.float32)

    cores = []
    cpb = 1
    for core in range(c.NC):
        em = np.nonzero(owner == core)[0]
        bucket = (pad_row[em] >= c.HALF).astype(np.int64)
        per_bucket = []
        for b in (0, 1):
            eb = em[bucket == b]
            eb = eb[np.argsort(dst_loc[eb], kind="stable")]
            blk = dst_loc[eb] // 128
            segs = []
            for bk in range(c.NBLK):
                run = eb[blk == bk]
                cpb = max(cpb, -(-len(run) // 128))
                segs.append((run, bk))
            per_bucket.append(segs)
        cores.append(per_bucket)

    # uniform cells: every (block, bucket) owns exactly cpb chunks so the
    # chunk -> dst-block map is compile-time static (SPMD-uniform).
    c.CPB = cpb
    EP = [c.NBLK * cpb * 128, c.NBLK * cpb * 128]
    EPT = EP[0] + EP[1]
    ZROW = c.NB  # all-zero table row (first pad node), same rel id both halves

    Wm = np.asarray(inputs["Wm"], np.float32)
    Ws = np.asarray(inputs["Ws"], np.float32)
    E = c.E
    Wef2 = np.concatenate([Wm[:, 2 * E:, :], Ws[:, 2 * E:, :]], axis=2)  # [L,41,128]

    in_maps = []
    for core in range(c.NC):
        srcrel = np.full(EPT, ZROW, np.int64)
        dstrel = np.full(EPT, ZROW, np.int64)
        dstblk = np.full(EPT, -1.0, np.float32)
        eperm = np.full(EPT, -1, np.int64)
        cell = cpb * 128
        for b in (0, 1):
            boff = b * EP[0]
            for run, bk in cores[core][b]:
                n = len(run)
                if n:
                    p0 = boff + bk * cell
                    sl = slice(p0, p0 + n)
                    srcrel[sl] = pad_row[run] - b * c.HALF
                    dstrel[sl] = dst_loc[run]
                    dstblk[sl] = (dst_loc[run] - bk * 128).astype(np.float32)
                    eperm[sl] = run

        eft = np.zeros((c.FE, EPT), np.float32)
        real = eperm >= 0
        eft[:, real] = ef[eperm[real]].T

        nfT = np.zeros((c.FV, c.NBP), np.float32)
        nfT[:, : c.NB] = nf[core * c.NB: (core + 1) * c.NB].T
        gidc = np.full(c.NBP, -1.0, np.float32)
        gidc[: c.NB] = gid[core * c.NB: (core + 1) * c.NB].astype(np.float32)

        # degree-weight columns for the exact sum(z) decomposition
        wsrc = np.zeros(c.NBP, np.float32)
        wsrc[: c.NB] = outdeg[core * c.NB: (core + 1) * c.NB]
        wdst = np.zeros(c.NBP, np.float32)
        wdst[: c.NB] = indeg[core * c.NB: (core + 1) * c.NB]

        # per-core edge-feature projection sums: (sum_e ef[e]) @ Wef2[l]
        efsum = ef[eperm[real]].sum(axis=0)  # [41]
        efW = np.stack([efsum @ Wef2[l] for l in range(c.L)], axis=0)  # [L,128]

        # dst-expansion indicator (node-partitioned): indT[d, pos] = 1 iff
        # edge at pos targets local row d within its (static) dst block.
        indT = (dstblk[None, :] == np.arange(128, dtype=np.float32)[:, None])
        m = {
            "gidx": _wrap_idx16(srcrel.astype(np.int16)),
            "indT": indT.astype(np.float16),
            "dstblk": dstblk.reshape(-1, 128).T.copy(),
            "eft": eft.astype(np.bfloat16) if hasattr(np, "bfloat16") else eft,
            "nfT": nfT,
            "gidc": gidc.reshape(-1, 128).T.copy(),
            "wsrc": wsrc.reshape(-1, 128).T.astype(np.float16).copy(),
            "wdst": wdst.reshape(-1, 128).T.astype(np.float16).copy(),
            "efW": efW.reshape(1, -1).astype(np.float32),
        }
        in_maps.append(m)

    def to_bf16(x):
        import ml_dtypes
        return x.astype(ml_dtypes.bfloat16)

    shared = {
        "W_emb": np.asarray(inputs["W_emb"], np.float32),
        "g_emb": np.asarray(inputs["g_emb"], np.float32).reshape(1, E),
        "be_emb": np.asarray(inputs["be_emb"], np.float32).reshape(1, E),
        "Wsrc2": to_bf16(np.concatenate([Wm[:, :E, :], Ws[:, :E, :]], axis=2)),
        "Wdst2": to_bf16(np.concatenate([Wm[:, E:2 * E, :], Ws[:, E:2 * E, :]], axis=2)),
        "Wef2": to_bf16(Wef2),
        "gm": np.asarray(inputs["gm"], np.float32),
        "bem": np.asarray(inputs["bem"], np.float32),
        "gs": np.asarray(inputs["gs"], np.float32),
        "bes": np.asarray(inputs["bes"], np.float32),
        "gn": np.asarray(inputs["gn"], np.float32),
        "ben": np.asarray(inputs["ben"], np.float32),
        "Wf0": np.asarray(inputs["Wf0"], np.float32),
        "gf0": np.asarray(inputs["gf0"], np.float32).reshape(-1, 1),
        "bef0": np.asarray(inputs["bef0"], np.float32).reshape(-1, 1),
        "Wf1": np.asarray(inputs["Wf1"], np.float32),
        "gf1": np.asarray(inputs["gf1"], np.float32).reshape(-1, 1),
        "bef1": np.asarray(inputs["bef1"], np.float32).reshape(-1, 1),
        "Wt": np.asarray(inputs["Wt"], np.float32),
        "bt": np.asarray(inputs["bt"], np.float32).reshape(1, 1),
    }
    for m in in_maps:
        # eft conversion (numpy lacks bfloat16; use ml_dtypes)
        m["eft"] = to_bf16(np.asarray(m["eft"], np.float32))
        m.update(shared)
    return in_maps, EP


# ------------------------------------------------------------- kernel build
def build(cfg, EP, dbg=False):
    c = cfg
    EPT = EP[0] + EP[1]
    NCH = EPT // 128
    DVE = mybir.EngineType.DVE
    AF = mybir.ActivationFunctionType
    OP = mybir.AluOpType

    nc = bacc.Bacc("TRN2", target_bir_lowering=False, debug=False,
                   enable_asserts=False, num_devices=c.NC, num_swdge_queues=4)

    def din(name, shape, dt=FP):
        return nc.dram_tensor(name, shape, dt, kind="ExternalInput")

    t_gidx = din("gidx", [128, EPT // 16], mybir.dt.int16)
    t_indT = din("indT", [128, EPT], F16)
    t_dstblk = din("dstblk", [128, NCH], FP)
    t_eft = din("eft", [c.FE, EPT], BF16)
    t_nfT = din("nfT", [c.FV, c.NBP])
    t_gidc = din("gidc", [128, c.NBLK])
    t_wsrc = din("wsrc", [128, c.NBLK], F16)
    t_wdst = din("wdst", [128, c.NBLK], F16)
    t_efW = din("efW", [1, c.L * c.ZF])
    t_Wemb = din("W_emb", [c.FV, c.E])
    t_gemb = din("g_emb", [1, c.E])
    t_beemb = din("be_emb", [1, c.E])
    t_Wsrc2 = din("Wsrc2", [c.L, c.E, c.ZF], BF16)
    t_Wdst2 = din("Wdst2", [c.L, c.E, c.ZF], BF16)
    t_Wef2 = din("Wef2", [c.L, c.FE, c.ZF], BF16)
    t_gm = din("gm", [c.L, c.E])
    t_bem = din("bem", [c.L, c.E])
    t_gs = din("gs", [c.L, c.E])
    t_bes = din("bes", [c.L, c.E])
    t_gn = din("gn", [c.L, c.E])
    t_ben = din("ben", [c.L, c.E])
    t_Wf0 = din("Wf0", [c.E, c.FC0])
    t_gf0 = din("gf0", [c.FC0, 1])
    t_bef0 = din("bef0", [c.FC0, 1])
    t_Wf1 = din("Wf1", [c.FC0, c.FC1])
    t_gf1 = din("gf1", [c.FC1, 1])
    t_bef1 = din("bef1", [c.FC1, 1])
    t_Wt = din("Wt", [c.E, 1])
    t_bt = din("bt", [1, 1])
    t_out = nc.dram_tensor("out", [1, c.NG], FP, kind="ExternalOutput")
    if dbg:
        t_dv = nc.dram_tensor("dbg_v", [128, c.NBP // 128 * c.E], FP, kind="ExternalOutput")
        t_dst0 = nc.dram_tensor("dbg_est0", [1, 2 * c.ZF], FP, kind="ExternalOutput")
        t_dagg = nc.dram_tensor("dbg_agg", [128, c.NBP // 128 * c.E], FP, kind="ExternalOutput")
        t_dvl = [nc.dram_tensor(f"dbg_vl{i}", [128, c.NBP // 128 * c.E], FP, kind="ExternalOutput")
                 for i in range(3)]
        t_dnst = [nc.dram_tensor(f"dbg_nst{i}", [1, 2 * c.E], FP, kind="ExternalOutput")
                  for i in range(3)]
        t_dz = nc.dram_tensor("dbg_z", [128, 4 * c.ZF], FP, kind="ExternalOutput")

    RG = [list(range(c.NC))]

    with tile.TileContext(nc) as tc, ExitStack() as es:
        dram = es.enter_context(tc.tile_pool(name="dram", bufs=1, space="DRAM"))
        zbuf = dram.tile([128, NCH, c.ZF], F16)
        est_in = [dram.tile([1, 2 * c.ZF], FP, name=f"est_in{i}") for i in range(c.L)]
        est_out = [dram.tile([1, 2 * c.ZF], FP, addr_space="Shared", name=f"est_out{i}")
                   for i in range(c.L)]
        nst_in = [dram.tile([1, 2 * c.E], FP, name=f"nst_in{i}") for i in range(c.L + 1)]
        nst_out = [dram.tile([1, 2 * c.E], FP, addr_space="Shared", name=f"nst_out{i}")
                   for i in range(c.L + 1)]
        agin_l = [dram.tile([c.NBP, c.ZF], F16, name=f"agin{i}") for i in range(c.L)]
        agout_l = [dram.tile([c.NT, c.ZF], F16, addr_space="Shared", name=f"agout{i}")
                   for i in range(c.L)]
        ro_in = dram.tile([c.E + 1, c.NG], FP)
        ro_out = dram.tile([c.E + 1, c.NG], FP, addr_space="Shared")

        konst = es.enter_context(tc.tile_pool(name="konst", bufs=1))
        iotaF = konst.tile([128, 256], FP)
        iota16 = konst.tile([128, 128], F16)
        identF = konst.tile([128, 128], FP)
        ones_row = konst.tile([1, 128], FP)
        ones_col = konst.tile([128, 1], FP)
        epsT = konst.tile([1, 1], FP)
        epsC = konst.tile([128, 1], FP)
        padmask = konst.tile([128, 1], FP)
        with tc.tile_pool(name="ksetup", bufs=1) as ks:
            ii = ks.tile([128, 256], mybir.dt.int32)
            nc.gpsimd.iota(ii[:], pattern=[[1, 256]], base=0, channel_multiplier=0)
            nc.vector.tensor_copy(iotaF[:], ii[:])
            nc.vector.tensor_copy(iota16[:], ii[:, :128])
            ip = ks.tile([128, 1], mybir.dt.int32)
            nc.gpsimd.iota(ip[:], pattern=[[1, 1]], base=0, channel_multiplier=1)
            ipf = ks.tile([128, 1], FP)
            nc.vector.tensor_copy(ipf[:], ip[:])
            nc.vector.tensor_scalar(identF[:], iotaF[:, :128], ipf[:], None, OP.is_equal)
            nc.vector.tensor_scalar(padmask[:], ipf[:], float(c.NB % 128), None, OP.is_lt)
        nc.vector.memset(ones_row[:], 1.0)
        nc.vector.memset(ones_col[:], 1.0)
        nc.vector.memset(epsT[:], c.EPS)
        nc.vector.memset(epsC[:], c.EPS)

        state = es.enter_context(tc.tile_pool(name="state", bufs=1))
        v_sb = state.tile([128, c.NBLK, c.E], FP)
        agg_sb = state.tile([128, c.NBLK, c.E], FP)
        dstblk_sb = state.tile([128, NCH], FP)
        gid_sb = state.tile([128, c.NBLK], FP)
        wsrc_sb = state.tile([128, c.NBLK], F16)
        wdst_sb = state.tile([128, c.NBLK], F16)
        efW_sb = state.tile([1, c.L * c.ZF], FP)
        sz_sb = state.tile([1, c.ZF], FP)
        s_g = state.tile([128, c.GS, c.ZF], F16)
        t_g = state.tile([128, c.GS, c.ZF], F16)
        asrc_sb = state.tile([128, c.NBLK, c.ZF], F16)
        adst_sb = state.tile([128, c.NBLK, c.ZF], F16)
        nc.sync.dma_start(dstblk_sb[:], t_dstblk[:])
        nc.sync.dma_start(gid_sb[:], t_gidc[:])
        nc.sync.dma_start(wsrc_sb[:], t_wsrc[:])
        nc.sync.dma_start(wdst_sb[:], t_wdst[:])
        nc.sync.dma_start(efW_sb[:], t_efW[:])

        wts = es.enter_context(tc.tile_pool(name="wts", bufs=1))
        Wsrc2_sb = wts.tile([c.E, c.L * c.ZF], BF16)
        Wdst2_sb = wts.tile([c.E, c.L * c.ZF], BF16)
        Wef2_sb = wts.tile([c.FE, c.L * c.ZF], BF16)
        for l in range(c.L):
            nc.sync.dma_start(Wsrc2_sb[:, l * c.ZF:(l + 1) * c.ZF], t_Wsrc2[l])
            nc.sync.dma_start(Wdst2_sb[:, l * c.ZF:(l + 1) * c.ZF], t_Wdst2[l])
            nc.sync.dma_start(Wef2_sb[:, l * c.ZF:(l + 1) * c.ZF], t_Wef2[l])

        def bn_fold(pool, sums, F, count, g_ap, be_ap):
            st = pool.tile([1, 2 * F], FP, name=f"bnf{nc.next_id()}")
            mean = pool.tile([1, F], FP, name=f"bnm{nc.next_id()}")
            var = pool.tile([1, F], FP, name=f"bnv{nc.next_id()}")
            nc.scalar.mul(mean[:], sums[:, 0:F], 1.0 / count)
            nc.scalar.mul(var[:], sums[:, F:2 * F], 1.0 / count)
            m2 = pool.tile([1, F], FP, name=f"bn2{nc.next_id()}")
            nc.vector.tensor_mul(m2[:], mean[:], mean[:])
            nc.vector.tensor_sub(var[:], var[:], m2[:])
            nc.scalar.activation(var[:], var[:], AF.Ln, bias=epsT[0:1, 0:1])
            nc.scalar.activation(var[:], var[:], AF.Exp, scale=-0.5)
            nc.vector.tensor_mul(st[:, 0:F], g_ap, var[:])
            nc.vector.tensor_mul(mean[:], mean[:], st[:, 0:F])
            nc.vector.tensor_sub(st[:, F:2 * F], be_ap, mean[:])
            return st

        def bcast_row(pool, psum_pool, row_ap, W, name):
            ps = psum_pool.tile([128, W], FP, name=f"ps{name}")
            nc.tensor.matmul(ps[:], ones_row[:, :], row_ap, start=True, stop=True)
            sb = pool.tile([128, W], FP, name=name)
            nc.scalar.copy(sb[:], ps[:])
            return sb

        def zero_vpad():
            cb = c.NB // 128
            nc.vector.tensor_scalar(v_sb[:, cb, :], v_sb[:, cb, :],
                                    padmask[:], None, OP.mult)

        # ---------------------------------------------------- embedding
        with tc.tile_pool(name="emb", bufs=1) as emb, \
             tc.tile_pool(name="embw", bufs=2) as embw, \
             tc.tile_pool(name="embp", bufs=2, space="PSUM") as embp, \
             tc.tile_pool(name="embs", bufs=1, space="PSUM") as embs:
            nfT_sb = emb.tile([c.FV, c.NBP], FP)
            nc.sync.dma_start(nfT_sb[:], t_nfT[:])
            Wemb_sb = emb.tile([c.FV, c.E], FP)
            nc.sync.dma_start(Wemb_sb[:], t_Wemb[:])
            z0 = emb.tile([128, c.NBLK, c.E], FP)
            ssum = embs.tile([1, c.E], FP)
            ssq = embs.tile([1, c.E], FP)
            for ch in range(c.NBLK):
                ps = embp.tile([128, c.E], FP, name="embz")
                nc.tensor.matmul(ps[:], nfT_sb[:, ch * 128:(ch + 1) * 128],
                                 Wemb_sb[:], start=True, stop=True)
                nc.scalar.copy(z0[:, ch, :], ps[:])
                sq = embw.tile([128, c.E], FP, name="embsq")
                nc.vector.tensor_mul(sq[:], z0[:, ch, :], z0[:, ch, :])
                nc.tensor.matmul(ssum[:], ones_col[:, :], z0[:, ch, :],
                                 start=(ch == 0), stop=(ch == c.NBLK - 1))
                nc.tensor.matmul(ssq[:], ones_col[:, :], sq[:],
                                 start=(ch == 0), stop=(ch == c.NBLK - 1))
            stat = emb.tile([1, 2 * c.E], FP)
            nc.vector.tensor_copy(stat[:, 0:c.E], ssum[:])
            nc.vector.tensor_copy(stat[:, c.E:], ssq[:])
            nc.sync.dma_start(nst_in[c.L][:], stat[:])
            nc.gpsimd.collective_compute(
                "AllReduce", OP.add, replica_groups=RG,
                ins=[nst_in[c.L].opt()], outs=[nst_out[c.L].opt()])
            rstat = emb.tile([1, 2 * c.E], FP)
            nc.sync.dma_start(rstat[:], nst_out[c.L][:])
            gemb_sb = emb.tile([1, c.E], FP)
            beemb_sb = emb.tile([1, c.E], FP)
            nc.sync.dma_start(gemb_sb[:], t_gemb[:])
            nc.sync.dma_start(beemb_sb[:], t_beemb[:])
            st = bn_fold(emb, rstat, c.E, c.N, gemb_sb[:], beemb_sb[:])
            stb = bcast_row(emb, embp, st[:], 2 * c.E, "embst")
            for ch in range(c.NBLK):
                u = embw.tile([128, c.E], FP, name="embu")
                nc.vector.tensor_mul(u[:], z0[:, ch, :], stb[:, 0:c.E])
                nc.vector.tensor_add(u[:], u[:], stb[:, c.E:])
                # silu(u) = u * exp(-ln(1 + exp(-u)))
                e = embw.tile([128, c.E], FP, name="embe")
                nc.scalar.activation(e[:], u[:], AF.Exp, scale=-1.0)
                nc.scalar.activation(e[:], e[:], AF.Ln, bias=1.0)
                nc.scalar.activation(e[:], e[:], AF.Exp, scale=-1.0)
                nc.vector.tensor_mul(v_sb[:, ch, :], u[:], e[:])
            zero_vpad()
        if dbg:
            nc.sync.dma_start(t_dv[:], v_sb[:].rearrange("p b f -> p (b f)"))

        # ---------------------------------------------------- conv layers
        gq = 0
        for l in range(c.L):
            # ---- phase A: projection tables (f16) + exact sum(z) dots
            with tc.tile_pool(name="phA", bufs=2) as pa, \
                 tc.tile_pool(name="phAp", bufs=2, space="PSUM") as pap, \
                 tc.tile_pool(name="phAo", bufs=2, space="PSUM") as pao, \
                 tc.tile_pool(name="phAs", bufs=1, space="PSUM") as pas:
                for ch in range(c.NBLK):
                    vt_ps = pap.tile([c.E, 128], FP, name="vtps")
                    nc.tensor.transpose(vt_ps[:], v_sb[:, ch, :], identF[:])
                    vt = pa.tile([c.E, 128], BF16, name="vt")
                    nc.scalar.copy(vt[:], vt_ps[:])
                    a1 = pao.tile([128, c.ZF], FP, name="a1")
                    nc.tensor.matmul(a1[:], vt[:], Wsrc2_sb[:, l * c.ZF:(l + 1) * c.ZF],
                                     start=True, stop=True)
                    nc.scalar.copy(asrc_sb[:, ch, :], a1[:])
                    a2 = pao.tile([128, c.ZF], FP, name="a2")
                    nc.tensor.matmul(a2[:], vt[:], Wdst2_sb[:, l * c.ZF:(l + 1) * c.ZF],
                                     start=True, stop=True)
                    nc.vector.tensor_copy(adst_sb[:, ch, :], a2[:])
                nc.sync.dma_start(
                    agin_l[l][:].rearrange("(b p) f -> p b f", p=128), asrc_sb[:])
                # exact sum(z): degree-weighted dots over local shards
                szsrc = pas.tile([1, c.ZF], FP)
                szdst = pas.tile([1, c.ZF], FP)
                for ch in range(c.NBLK):
                    nc.tensor.matmul(szsrc[:], wsrc_sb[:, ch:ch + 1],
                                     asrc_sb[:, ch, :],
                                     start=(ch == 0), stop=(ch == c.NBLK - 1))
                    nc.tensor.matmul(szdst[:], wdst_sb[:, ch:ch + 1],
                                     adst_sb[:, ch, :],
                                     start=(ch == 0), stop=(ch == c.NBLK - 1))
                nc.vector.tensor_copy(sz_sb[:], szsrc[:])
                nc.vector.tensor_add(sz_sb[:], sz_sb[:], szdst[:])
                nc.vector.tensor_add(sz_sb[:], sz_sb[:],
                                     efW_sb[:, l * c.ZF:(l + 1) * c.ZF])
            nc.gpsimd.collective_compute(
                "AllGather", OP.bypass, replica_groups=RG,
                ins=[agin_l[l].opt()], outs=[agout_l[l].opt()])


            # ---- pass 1: z assembly + f16 spill + Gram stats
            with tc.tile_pool(name="p1idx", bufs=4) as pidx, \
                 tc.tile_pool(name="p1g", bufs=4) as pg, \
                 tc.tile_pool(name="p1z", bufs=3) as pz, \
                 tc.tile_pool(name="p1st", bufs=1) as p1st, \
                 tc.tile_pool(name="p1zp", bufs=1, space="PSUM") as pzp, \
                 tc.tile_pool(name="p1gr", bufs=1, space="PSUM") as pgr:
                gram = pgr.tile([128, c.ZF], FP)
                first_mm = [True]
                nchb = EP[0] // 128   # chunks per bucket (== NBLK * CPB)
                for b in (0, 1):
                    base_ch = b * nchb
                    for g0 in range(0, nchb, c.GS):
                        gs = min(c.GS, nchb - g0)
                        ni = gs * 128
                        coff = base_ch + g0
                        last_grp = (b == 1 and g0 + c.GS >= nchb)
                        idxs_t = pidx.tile([128, c.GS * 8], mybir.dt.int16, name="idxs")
                        nc.sync.dma_start(idxs_t[:, :gs * 8],
                                          t_gidx[:, coff * 8:coff * 8 + gs * 8])
                        gath = pg.tile([128, c.GS, c.ZF], F16, name="gath")
                        nc.gpsimd.dma_gather(
                            gath[:, :gs, :],
                            agout_l[l][b * c.HALF:(b + 1) * c.HALF, :],
                            idxs_t[:, :gs * 8], num_idxs=ni, num_idxs_reg=ni,
                            elem_size=c.ZF, queue_num=gq % 4, single_packet=False)
                        gq += 1
                        indT_t = pg.tile([128, c.GS, 128], F16, name="indT")
                        nc.sync.dma_start(
                            indT_t[:, :gs, :],
                            t_indT[:, coff * 128:coff * 128 + ni]
                            .rearrange("p (g e) -> p g e", e=128))
                        ef_t = pg.tile([c.FE, c.GS * 128], BF16, name="eft")
                        nc.sync.dma_start(ef_t[:, :ni],
                                          t_eft[:, coff * 128:coff * 128 + ni])
                        zp = pzp.tile([128, c.GS, c.ZF], FP, name="zp")
                        for j in range(gs):
                            blk = (g0 + j) // c.CPB
                            nc.tensor.matmul(zp[:, j, :], ef_t[:, j * 128:(j + 1) * 128],
                                             Wef2_sb[:, l * c.ZF:(l + 1) * c.ZF],
                                             start=True, stop=False)
                            nc.tensor.matmul(zp[:, j, :], indT_t[:, j, :],
                                             adst_sb[:, blk, :],
                                             start=False, stop=True)
                        pef_t = pz.tile([128, c.GS, c.ZF], F16, name="peft")
                        nc.scalar.copy(pef_t[:, :gs, :], zp[:, :gs, :])
                        z_t = pz.tile([128, c.GS, c.ZF], F16, name="zt")
                        nc.vector.tensor_add(z_t[:, :gs, :], gath[:, :gs, :],
                                             pef_t[:, :gs, :])
                        nc.sync.dma_start(zbuf[:, coff:coff + gs, :], z_t[:, :gs, :])
                        for j in range(gs):
                            nc.tensor.matmul(gram[:], z_t[:, j, :], z_t[:, j, :],
                                             start=first_mm[0],
                                             stop=(last_grp and j == gs - 1))
                            first_mm[0] = False
                # stats: diag(gram) -> row; assemble [sum(z) | sum(z^2)]
                gI = p1st.tile([128, c.ZF], FP)
                nc.vector.tensor_mul(gI[:], gram[:], identF[:])
                with tc.tile_pool(name="p1sq", bufs=1, space="PSUM") as psq:
                    szq = psq.tile([1, c.ZF], FP)
                    nc.tensor.matmul(szq[:], ones_col[:, :], gI[:], start=True, stop=True)
                    stat = p1st.tile([1, 2 * c.ZF], FP)
                    nc.vector.tensor_copy(stat[:, :c.ZF], sz_sb[:])
                    nc.vector.tensor_copy(stat[:, c.ZF:], szq[:])
                    nc.sync.dma_start(est_in[l][:], stat[:])

            nc.gpsimd.collective_compute(
                "AllReduce", OP.add, replica_groups=RG,
                ins=[est_in[l].opt()], outs=[est_out[l].opt()])

            # ---- fold BN affine, broadcast s/t to [128, GS, ZF] f16
            with tc.tile_pool(name="pmid", bufs=1) as pm, \
                 tc.tile_pool(name="pmidp", bufs=1, space="PSUM") as pmp:
                rstat = pm.tile([1, 2 * c.ZF], FP)
                nc.sync.dma_start(rstat[:], est_out[l][:])
                gms = pm.tile([1, 2 * c.E], FP)
                nc.sync.dma_start(gms[:, :c.E], t_gm[l:l + 1, :])
                nc.sync.dma_start(gms[:, c.E:], t_gs[l:l + 1, :])
                bms = pm.tile([1, 2 * c.E], FP)
                nc.sync.dma_start(bms[:, :c.E], t_bem[l:l + 1, :])
                nc.sync.dma_start(bms[:, c.E:], t_bes[l:l + 1, :])
                if dbg and l == 0:
                    nc.sync.dma_start(t_dst0[:], rstat[:])
                    zdbg = pm.tile([128, 4, c.ZF], F16, name="zdbg")
                    nc.sync.dma_start(zdbg[:], zbuf[:, 0:4, :])
                    zdbgf = pm.tile([128, 4 * c.ZF], FP, name="zdbgf")
                    nc.vector.tensor_copy(zdbgf[:], zdbg[:].rearrange("p b f -> p (b f)"))
                    nc.sync.dma_start(t_dz[:], zdbgf[:])
                st = bn_fold(pm, rstat, c.ZF, c.M, gms[:], bms[:])
                stb = bcast_row(pm, pmp, st[:], 2 * c.ZF, "edgest")
                # negate the sigmoid (m) half so exp(u) = exp(-um) there
                nc.vector.tensor_scalar(stb[:, 0:c.E], stb[:, 0:c.E],
                                        -1.0, None, OP.mult)
                nc.vector.tensor_scalar(stb[:, c.ZF:c.ZF + c.E],
                                        stb[:, c.ZF:c.ZF + c.E],
                                        -1.0, None, OP.mult)
                for j in range(c.GS):
                    nc.vector.tensor_copy(s_g[:, j, :], stb[:, 0:c.ZF])
                    nc.vector.tensor_copy(t_g[:, j, :], stb[:, c.ZF:])

            # ---- pass 2: activations + PSUM-resident scatter
            with tc.tile_pool(name="p2z", bufs=3) as p2z, \
                 tc.tile_pool(name="p2ap", bufs=1, space="PSUM") as p2ap:
                agg_ps = p2ap.tile([128, c.NBLK, c.E], FP)
                nc.vector.memset(agg_ps[:], 0.0)
                n_sc = [0]
                nchb = EP[0] // 128
                for b in (0, 1):
                    base_ch = b * nchb
                    for g0 in range(0, nchb, c.GS):
                        gs = min(c.GS, nchb - g0)
                        coff = base_ch + g0
                        z_t = p2z.tile([128, c.GS, c.ZF], F16, name="z2t")
                        nc.sync.dma_start(z_t[:, :gs, :], zbuf[:, coff:coff + gs, :])
                        u = p2z.tile([128, c.GS, c.ZF], F16, name="u")
                        nc.vector.tensor_mul(u[:, :gs, :], z_t[:, :gs, :], s_g[:, :gs, :])
                        nc.vector.tensor_add(u[:, :gs, :], u[:, :gs, :], t_g[:, :gs, :])
                        uc = p2z.tile([128, c.GS, c.ZF], F16, name="uc")
                        nc.vector.tensor_scalar_min(uc[:, :gs, :], u[:, :gs, :], 11.0)
                        e_t = p2z.tile([128, c.GS, c.ZF], F16, name="et")
                        nc.scalar.activation(e_t[:, :gs, :], uc[:, :gs, :], AF.Exp)
                        lnm = p2z.tile([128, c.GS, c.E], F16, name="lnm")
                        nc.scalar.activation(lnm[:, :gs, :], e_t[:, :gs, 0:c.E],
                                             AF.Ln, bias=1.0)
                        sg_t = p2z.tile([128, c.GS, c.E], F16, name="sgt")
                        nc.scalar.activation(sg_t[:, :gs, :], lnm[:, :gs, :],
                                             AF.Exp, scale=-1.0)
                        sp_t = p2z.tile([128, c.GS, c.E], F16, name="spt")
                        nc.scalar.activation(sp_t[:, :gs, :], e_t[:, :gs, c.E:],
                                             AF.Ln, bias=1.0)
                        # softplus(u) == u at f16 precision for u > 11 (clamped above)
                        nc.vector.tensor_tensor(sp_t[:, :gs, :], sp_t[:, :gs, :],
                                                u[:, :gs, c.E:], OP.max)
                        h = p2z.tile([128, c.GS, c.E], F16, name="h")
                        nc.vector.tensor_mul(h[:, :gs, :], sg_t[:, :gs, :],
                                             sp_t[:, :gs, :])
                        ind_t = p2z.tile([128, c.GS, 128], F16, name="ind")
                        for j in range(gs):
                            ch = coff + j
                            cb = g0 + j           # chunk index within bucket
                            blk = cb // c.CPB
                            pos = cb % c.CPB
                            nc.vector.tensor_scalar(ind_t[:, j, :], iota16[:, :],
                                                    dstblk_sb[:, ch:ch + 1], None,
                                                    OP.is_equal)
                            n_sc[0] += 1
                            nc.tensor.matmul(
                                agg_ps[:, blk, :],
                                ind_t[:, j, :], h[:, j, :],
                                start=False, stop=(n_sc[0] == NCH),
                                skip_group_check=True)
                # drain agg PSUM -> SBUF
                agg_flat = agg_ps[:].rearrange("p b f -> p (b f)")
                for o in range(0, c.NBLK * c.E, 512):
                    w = min(512, c.NBLK * c.E - o)
                    nc.scalar.copy(
                        agg_sb[:].rearrange("p b f -> p (b f)")[:, o:o + w],
                        agg_flat[:, o:o + w])

            if dbg and l == 0:
                nc.sync.dma_start(t_dagg[:], agg_sb[:].rearrange("p b f -> p (b f)"))

            # ---- node BN + update
            with tc.tile_pool(name="nod", bufs=1) as nod, \
                 tc.tile_pool(name="nodw", bufs=2) as nodw, \
                 tc.tile_pool(name="nodp", bufs=2, space="PSUM") as nodp, \
                 tc.tile_pool(name="nods", bufs=1, space="PSUM") as nods:
                nsum = nods.tile([1, c.E], FP)
                nssq = nods.tile([1, c.E], FP)
                for ch in range(c.NBLK):
                    sq = nodw.tile([128, c.E], FP, name="nsq")
                    nc.vector.tensor_mul(sq[:], agg_sb[:, ch, :], agg_sb[:, ch, :])
                    nc.tensor.matmul(nsum[:], ones_col[:, :], agg_sb[:, ch, :],
                                     start=(ch == 0), stop=(ch == c.NBLK - 1))
                    nc.tensor.matmul(nssq[:], ones_col[:, :], sq[:],
                                     start=(ch == 0), stop=(ch == c.NBLK - 1))
                stat = nod.tile([1, 2 * c.E], FP)
                nc.vector.tensor_copy(stat[:, :c.E], nsum[:])
                nc.vector.tensor_copy(stat[:, c.E:], nssq[:])
                nc.sync.dma_start(nst_in[l][:], stat[:])
                nc.gpsimd.collective_compute(
                    "AllReduce", OP.add, replica_groups=RG,
                    ins=[nst_in[l].opt()], outs=[nst_out[l].opt()])
                rstat = nod.tile([1, 2 * c.E], FP)
                nc.sync.dma_start(rstat[:], nst_out[l][:])
                gn_sb = nod.tile([1, c.E], FP)
                ben_sb = nod.tile([1, c.E], FP)
                nc.sync.dma_start(gn_sb[:], t_gn[l:l + 1, :])
                nc.sync.dma_start(ben_sb[:], t_ben[l:l + 1, :])
                st = bn_fold(nod, rstat, c.E, c.N, gn_sb[:], ben_sb[:])
                stb = bcast_row(nod, nodp, st[:], 2 * c.E, "nodst")
                for ch in range(c.NBLK):
                    u = nodw.tile([128, c.E], FP, name="nu")
                    nc.vector.tensor_mul(u[:], agg_sb[:, ch, :], stb[:, 0:c.E])
                    nc.vector.tensor_add(u[:], u[:], stb[:, c.E:])
                    nc.vector.tensor_add(u[:], u[:], v_sb[:, ch, :])
                    # softplus(u) = ln(1 + exp(u)); u is comfortably < 80
                    e = nodw.tile([128, c.E], FP, name="ne")
                    nc.scalar.activation(e[:], u[:], AF.Exp)
                    nc.scalar.activation(v_sb[:, ch, :], e[:], AF.Ln, bias=1.0)
                zero_vpad()
                if dbg:
                    nc.sync.dma_start(t_dvl[l][:], v_sb[:].rearrange("p b f -> p (b f)"))
                    nc.sync.dma_start(t_dnst[l][:], rstat[:])

        # ---------------------------------------------------- readout
        with tc.tile_pool(name="ro", bufs=1) as ro, \
             tc.tile_pool(name="row", bufs=2) as row, \
             tc.tile_pool(name="rop", bufs=1, space="PSUM") as rop, \
             tc.tile_pool(name="ros", bufs=1, space="PSUM") as ros:
            psums = ros.tile([c.E, c.NG], FP)
            pcnt = ros.tile([1, c.NG], FP)
            for ch in range(c.NBLK):
                gind = row.tile([128, c.NG], FP, name="gind")
                nc.vector.tensor_scalar(gind[:], iotaF[:, :c.NG],
                                        gid_sb[:, ch:ch + 1], None, OP.is_equal)
                nc.tensor.matmul(psums[:], v_sb[:, ch, :], gind[:],
                                 start=(ch == 0), stop=(ch == c.NBLK - 1))
                nc.tensor.matmul(pcnt[:], ones_col[:, :], gind[:],
                                 start=(ch == 0), stop=(ch == c.NBLK - 1))
            acc = ro.tile([c.E + 1, c.NG], FP)
            nc.scalar.copy(acc[0:c.E, :], psums[:])
            nc.scalar.copy(acc[c.E:c.E + 1, :], pcnt[:])
            nc.sync.dma_start(ro_in[:], acc[:])
            nc.gpsimd.collective_compute(
                "AllReduce", OP.add, replica_groups=RG,
                ins=[ro_in.opt()], outs=[ro_out.opt()])
            racc = ro.tile([c.E + 1, c.NG], FP)
            nc.sync.dma_start(racc[:], ro_out[:])
            cnt = ro.tile([1, c.NG], FP)
            nc.vector.tensor_scalar_max(cnt[:], racc[c.E:c.E + 1, :], 1.0)
            nc.scalar.activation(cnt[:], cnt[:], AF.Ln)
            nc.scalar.activation(cnt[:], cnt[:], AF.Exp, scale=-1.0)
            rcb_ps = rop.tile([c.E, c.NG], FP, name="rcb", tag="rosc")
            nc.tensor.matmul(rcb_ps[:], ones_row[:, 0:c.E], cnt[:], start=True, stop=True)
            vs = ro.tile([c.E, c.NG], FP)
            nc.vector.tensor_tensor(vs[:], racc[0:c.E, :], rcb_ps[:], OP.mult)

            def fc_bn_silu(pool, psum_pool, x_sb, W_ap, K, Fo, g_t, be_t, nm):
                ps = psum_pool.tile([Fo, c.NG], FP, name=f"fc{nm}", tag="rosc")
                W_sb = pool.tile([K, Fo], FP, name=f"W{nm}")
                nc.sync.dma_start(W_sb[:], W_ap)
                nc.tensor.matmul(ps[:], W_sb[:], x_sb[:], start=True, stop=True)
                g_sb = pool.tile([Fo, 1], FP, name=f"g{nm}")
                be_sb = pool.tile([Fo, 1], FP, name=f"be{nm}")
                nc.sync.dma_start(g_sb[:], g_t[:])
                nc.sync.dma_start(be_sb[:], be_t[:])
                x_sbc = pool.tile([Fo, c.NG], FP, name=f"x{nm}")
                nc.scalar.copy(x_sbc[:], ps[:])
                sums = pool.tile([Fo, 1], FP, name=f"su{nm}")
                nc.vector.tensor_reduce(sums[:], x_sbc[:], mybir.AxisListType.X, OP.add)
                sq = pool.tile([Fo, c.NG], FP, name=f"sq{nm}")
                nc.vector.tensor_mul(sq[:], x_sbc[:], x_sbc[:])
                ssq = pool.tile([Fo, 1], FP, name=f"sl{nm}")
                nc.vector.tensor_reduce(ssq[:], sq[:], mybir.AxisListType.X, OP.add)
                mean = pool.tile([Fo, 1], FP, name=f"mn{nm}")
                nc.scalar.mul(mean[:], sums[:], 1.0 / c.NG)
                var = pool.tile([Fo, 1], FP, name=f"vr{nm}")
                nc.scalar.mul(var[:], ssq[:], 1.0 / c.NG)
                m2 = pool.tile([Fo, 1], FP, name=f"m2{nm}")
                nc.vector.tensor_mul(m2[:], mean[:], mean[:])
                nc.vector.tensor_sub(var[:], var[:], m2[:])
                nc.scalar.activation(var[:], var[:], AF.Ln, bias=epsC[0:Fo, :])
                nc.scalar.activation(var[:], var[:], AF.Exp, scale=-0.5)
                s_col = pool.tile([Fo, 1], FP, name=f"sc{nm}")
                nc.vector.tensor_mul(s_col[:], g_sb[:], var[:])
                t_col = pool.tile([Fo, 1], FP, name=f"tc{nm}")
                nc.vector.tensor_mul(t_col[:], mean[:], s_col[:])
                nc.vector.tensor_sub(t_col[:], be_sb[:], t_col[:])
                u = pool.tile([Fo, c.NG], FP, name=f"u{nm}")
                nc.scalar.activation(u[:], x_sbc[:], AF.Identity,
                                     bias=t_col[:], scale=s_col[:])
                # silu(u) = u * exp(-ln(1 + exp(-u)))
                e = pool.tile([Fo, c.NG], FP, name=f"e{nm}")
                nc.scalar.activation(e[:], u[:], AF.Exp, scale=-1.0)
                nc.scalar.activation(e[:], e[:], AF.Ln, bias=1.0)
                nc.scalar.activation(e[:], e[:], AF.Exp, scale=-1.0)
                out = pool.tile([Fo, c.NG], FP, name=f"o{nm}")
                nc.vector.tensor_mul(out[:], u[:], e[:])
                return out

            z1 = fc_bn_silu(ro, rop, vs, t_Wf0[:], c.E, c.FC0, t_gf0, t_bef0, "0")
            z2 = fc_bn_silu(ro, rop, z1, t_Wf1[:], c.FC0, c.FC1, t_gf1, t_bef1, "1")
            Wt_sb = ro.tile([c.E, 1], FP)
            nc.sync.dma_start(Wt_sb[:], t_Wt[:])
            hd = rop.tile([1, c.NG], FP, name="hd", tag="rosc")
            nc.tensor.matmul(hd[:], Wt_sb[:], z2[:], start=True, stop=True)
            bt_sb = ro.tile([1, 1], FP)
            nc.sync.dma_start(bt_sb[:], t_bt[:])
            res = ro.tile([1, c.NG], FP)
            nc.vector.tensor_scalar(res[:], hd[:], bt_sb[0:1, 0:1], None, OP.add)
            nc.sync.dma_start(t_out[:], res[:])

    nc.compile()
    return nc


# ------------------------------------------------------------------ driver
_CACHE = {}


def kernel(**inputs):
    cfg = Cfg(int(inputs["node_feats"].shape[0]),
              int(inputs["src"].shape[0]), 256)
    in_maps, EP = preprocess(inputs, cfg)
    key = (cfg.N, cfg.M, tuple(EP))
    if key not in _CACHE:
        _CACHE[key] = build(cfg, EP)
    nc = _CACHE[key]
    res = bass_utils.run_bass_kernel_spmd(
        nc, in_maps, core_ids=list(range(cfg.NC)), trace=False)
    out = np.asarray(res.results[0]["out"], np.float32)
    return out.reshape(cfg.NG, 1)
